# revision 40
# baseline (speedup 1.0000x reference)
"""TRN2 Bass kernel for the two-encoder attention module.

Per batch element b (8 of them, one per NeuronCore):
    P1 = X @ E1^T          (T,S)   attention logits vs `sent`
    A1 = softmax(P1)               -> output sent_weight
    C1 = A1 @ E1           (T,H)
    P2/A2/C2 vs `template` (St)
    gate = sigmoid(cat(C1, C2, X) @ W_gate^T)        (T,H)
    fusion = tanh((1-gate)*C1@Ws^T + gate*C2@Wt^T + X@Wo^T)

Everything on-device is computed in a feature-major ("transposed") layout so
that every matmul contraction dim lands on SBUF partitions with no on-device
input transposes:
    - logits: lhsT = X^T (hi/lo bf16 split, 3-pass for fp32-grade accuracy),
      rhs = E^T (hi/lo).  Softmax along the free dim.
    - A^T obtained with PE transpose-mode matmuls (f16).
    - C^T  = E^T(natural lhsT) @ A^T    [f16]
    - gate^T, F^T via host-pretransposed/prepacked f16 weights.
Host side handles batch sharding across 8 cores, input transposes/casts and
the final un-transpose of fusion.
"""
import contextlib
import os

import ml_dtypes
import numpy as np

B, T, S, St, H = 8, 1024, 1024, 512, 1024
TB = T // 128           # 8 t-tiles
HB = H // 128           # 8 h-blocks
SB = S // 128           # 8 s-blocks (sent)
S2B = St // 128         # 4 s-blocks (template)
NT = T // 512           # 2 t-halves for 512-wide rhs

_CACHE = {}


def _build(reps=1, bc="f16"):
    import concourse.bacc as bacc
    import concourse.mybir as mybir
    import concourse.tile as tile

    dt = mybir.dt
    F32, F16, BF16 = dt.float32, dt.float16, dt.bfloat16
    if bc == "bf16":
        F16 = BF16  # dtype for the value/gate/fusion stages
    AF = mybir.ActivationFunctionType
    ALU = mybir.AluOpType
    AX = mybir.AxisListType

    nc = bacc.Bacc("TRN2", target_bir_lowering=False, debug=False, num_devices=8)

    d = {}
    for nm, shape, ddt in [
        ("xt_hi", [H, T], BF16), ("xt_lo", [H, T], BF16),
        ("e1t_hi", [H, S], BF16), ("e1t_lo", [H, S], BF16),
        ("e2t_hi", [H, St], BF16), ("e2t_lo", [H, St], BF16),
        ("e1f", [S, H], F16), ("e2f", [St, H], F16), ("xtf", [H, T], F16),
        ("wg", [H, 3 * H], F16), ("ws", [H, H], F16),
        ("wt", [H, H], F16), ("wo", [H, H], F16),
        ("ident", [128, 128], F16),
    ]:
        d[nm] = nc.dram_tensor(nm, shape, ddt, kind="ExternalInput")
    d["aw1"] = nc.dram_tensor("aw1", [T, S], F32, kind="ExternalOutput")
    d["aw2"] = nc.dram_tensor("aw2", [T, St], F32, kind="ExternalOutput")
    d["fusT"] = nc.dram_tensor("fusT", [H, T], F32, kind="ExternalOutput")

    with tile.TileContext(nc) as tc, contextlib.ExitStack() as ctx:
        P = ctx.enter_context(tc.tile_pool(name="persist", bufs=1))

        ident_t = P.tile([128, 128], F16, tag="ident")
        nc.gpsimd.dma_start(ident_t[:], d["ident"].ap()[:, :])

        def load_blocked(tag, dram, nblk, width, ddt, eng=None, nsplit=1):
            t = P.tile([128, nblk * width], ddt, tag=tag)
            eng = eng or nc.sync
            cw = width // nsplit
            for sp in range(nsplit):
                for b_ in range(nblk):
                    eng.dma_start(
                        t[:, b_ * width + sp * cw: b_ * width + (sp + 1) * cw],
                        dram.ap()[b_ * 128:(b_ + 1) * 128, sp * cw:(sp + 1) * cw])
            return t

        pending = []  # (a16 tile, it, nsb, a_t) transposes deferred one block

        def attention(it_range, nsb, xt_hi_t, xt_lo_t, et_hi_t, et_lo_t, a_t,
                      aw_dram, PSA, PST, WRK, STT, width):
            """One encoder's logits+softmax+transpose. width = S or St.

            Transposes for tile `it` are emitted after the logits of `it+1`
            so the softmax chain (DVE/ACT) has a full logits-block of slack
            before the PE reaches the transpose instructions.
            """
            nhalf = width // 512

            def flush_pending():
                for a16p, itp, nsbp, a_tp in pending:
                    for sb in range(nsbp):
                        pt = PST.tile([128, 128], F16, tag="tr")
                        nc.tensor.transpose(
                            pt[:], a16p[:, sb * 128:(sb + 1) * 128], ident_t[:])
                        nc.vector.tensor_copy(
                            a_tp[:, sb * T + itp * 128: sb * T + itp * 128 + 128],
                            pt[:])
                pending.clear()

            for it in it_range:
                # s-halves interleaved so consecutive matmuls share lhsT
                ps = [PSA.tile([128, 512], F32, tag="p", name=f"p{sh}")
                      for sh in range(nhalf)]
                i = 0
                for lt, rt in ((xt_hi_t, et_hi_t), (xt_hi_t, et_lo_t),
                               (xt_lo_t, et_hi_t)):
                    for hb in range(HB):
                        lhsT = lt[:, hb * T + it * 128: hb * T + it * 128 + 128]
                        for sh in range(nhalf):
                            nc.tensor.matmul(
                                ps[sh][:],
                                lhsT,
                                rt[:, hb * width + sh * 512: hb * width + sh * 512 + 512],
                                start=(i < nhalf), stop=(i >= (3 * HB - 1) * nhalf))
                            i += 1
                flush_pending()
                nm = STT.tile([128, 1], F32, tag="nm")
                if nhalf == 1:
                    nc.vector.tensor_reduce(nm[:], ps[0][:], axis=AX.X,
                                            op=ALU.max, negate=True)
                else:
                    nm0 = STT.tile([128, 1], F32, tag="nm0")
                    nm1 = STT.tile([128, 1], F32, tag="nm1")
                    nc.vector.tensor_reduce(nm0[:], ps[0][:], axis=AX.X,
                                            op=ALU.max, negate=True)
                    nc.vector.tensor_reduce(nm1[:], ps[1][:], axis=AX.X,
                                            op=ALU.max, negate=True)
                    nc.vector.tensor_tensor(nm[:], nm0[:], nm1[:], op=ALU.min)
                a_f32 = WRK.tile([128, width], F32, tag="a_f32")
                ssum = STT.tile([128, 1], F32, tag="ssum")
                if nhalf == 1:
                    nc.scalar.activation(a_f32[:], ps[0][:], AF.Exp,
                                         bias=nm[:], scale=1.0, accum_out=ssum[:])
                else:
                    s0 = STT.tile([128, 1], F32, tag="s0")
                    s1 = STT.tile([128, 1], F32, tag="s1")
                    nc.scalar.activation(a_f32[:, 0:512], ps[0][:], AF.Exp,
                                         bias=nm[:], scale=1.0, accum_out=s0[:])
                    nc.scalar.activation(a_f32[:, 512:1024], ps[1][:], AF.Exp,
                                         bias=nm[:], scale=1.0, accum_out=s1[:])
                    nc.vector.tensor_add(ssum[:], s0[:], s1[:])
                rinv = STT.tile([128, 1], F32, tag="rinv")
                nc.vector.reciprocal(rinv[:], ssum[:])
                an = WRK.tile([128, width], F32, tag="an")
                nc.scalar.activation(an[:], a_f32[:], AF.Copy, scale=rinv[:])
                nc.sync.dma_start(aw_dram.ap()[it * 128:(it + 1) * 128, :], an[:])
                a16 = WRK.tile([128, width], F16, tag="a16")
                nc.vector.tensor_scalar_mul(a16[:], a_f32[:], rinv[:])
                pending.append((a16, it, nsb, a_t))
            return flush_pending

        def ctx_matmul(c_t, ef_t, a_t, nsb, PSA):
            """C^T[k,t] = sum_s E[s,k] * A^T[s,t]. t-halves share lhsT."""
            for kt in range(HB):
                pcs = [PSA.tile([128, 512], F32, tag="p", name=f"pc{tb}")
                       for tb in range(NT)]
                for sb in range(nsb):
                    lhsT = ef_t[:, sb * H + kt * 128: sb * H + kt * 128 + 128]
                    for tb in range(NT):
                        nc.tensor.matmul(
                            pcs[tb][:], lhsT,
                            a_t[:, sb * T + tb * 512: sb * T + tb * 512 + 512],
                            start=(sb == 0), stop=(sb == nsb - 1))
                for tb in range(NT):
                    nc.scalar.copy(
                        c_t[:, kt * T + tb * 512: kt * T + tb * 512 + 512],
                        pcs[tb][:])

        for _rep in range(reps):
            pending.clear()
            # critical-path loads first: t-tile 0 logits need all of xt_hi and
            # e1t_hi; xt_lo/e1t_lo are needed one matmul-group later.
            xt_hi_t = load_blocked("xt_hi", d["xt_hi"], HB, T, BF16, nc.sync)
            xt_lo_t = load_blocked("xt_lo", d["xt_lo"], HB, T, BF16, nc.scalar)
            e1t_hi_t = load_blocked("e1t_hi", d["e1t_hi"], HB, S, BF16, nc.sync)
            e1t_lo_t = load_blocked("e1t_lo", d["e1t_lo"], HB, S, BF16, nc.scalar)
            a1t_t = P.tile([128, SB * T], F16, tag="a1t")
            a2t_t = P.tile([128, S2B * T], F16, tag="a2t")

            with tc.tile_pool(name="psA", bufs=6, space="PSUM") as PSA, \
                 tc.tile_pool(name="psT", bufs=2, space="PSUM") as PST, \
                 tc.tile_pool(name="wrkA", bufs=2) as WRK, \
                 tc.tile_pool(name="stats", bufs=3) as STT:
                # sent attention, then template attention (fills PE while the
                # tail of sent softmax/transposes completes), then both C^T.
                attention(range(TB), SB, xt_hi_t, xt_lo_t, e1t_hi_t, e1t_lo_t,
                          a1t_t, d["aw1"], PSA, PST, WRK, STT, S)
                e2t_hi_t = load_blocked("e2t_hi", d["e2t_hi"], HB, St, BF16,
                                        nc.gpsimd)
                e2t_lo_t = load_blocked("e2t_lo", d["e2t_lo"], HB, St, BF16,
                                        nc.gpsimd)
                e1f_t = load_blocked("e1f", d["e1f"], SB, H, F16, nc.gpsimd)
                e2f_t = load_blocked("e2f", d["e2f"], S2B, H, F16, nc.gpsimd)
                flush_last = attention(
                    range(TB), S2B, xt_hi_t, xt_lo_t, e2t_hi_t, e2t_lo_t,
                    a2t_t, d["aw2"], PSA, PST, WRK, STT, St)
                flush_last()
                c1t_t = P.tile([128, HB * T], F16, tag="e1t_hi")  # slot reuse
                ctx_matmul(c1t_t, e1f_t, a1t_t, SB, PSA)
                c2t_t = P.tile([128, HB * T], F16, tag="e1t_lo")  # slot reuse
                ctx_matmul(c2t_t, e2f_t, a2t_t, S2B, PSA)

            # fused gate + fusion stage (all f16 matmuls)
            xtf_t = load_blocked("xt_hi", d["xtf"], HB, T, F16)  # slot reuse
            with tc.tile_pool(name="psC", bufs=2, space="PSUM") as PSC, \
                 tc.tile_pool(name="wrkC", bufs=2) as WC:
                for ht in range(HB):
                    wgt = WC.tile([128, 24 * 128], F16, tag="wg")
                    nc.sync.dma_start(wgt[:], d["wg"].ap()[ht * 128:(ht + 1) * 128, :])
                    wsto = WC.tile([128, 3 * 8 * 128], F16, tag="wsto")
                    for j, wnm in enumerate(("ws", "wt", "wo")):
                        nc.sync.dma_start(
                            wsto[:, j * 1024:(j + 1) * 1024],
                            d[wnm].ap()[ht * 128:(ht + 1) * 128, :])
                    for tb in range(NT):
                        pg = PSC.tile([128, 512], F32, tag="g")
                        i = 0
                        for src in (c1t_t, c2t_t, xtf_t):
                            for kb in range(HB):
                                nc.tensor.matmul(
                                    pg[:], wgt[:, i * 128:(i + 1) * 128],
                                    src[:, kb * T + tb * 512: kb * T + tb * 512 + 512],
                                    start=(i == 0), stop=(i == 23))
                                i += 1
                        pf = []
                        for j, src in enumerate((c1t_t, c2t_t, xtf_t)):
                            pfj = PSC.tile([128, 512], F32, tag=f"f{j}")
                            for kb in range(HB):
                                nc.tensor.matmul(
                                    pfj[:],
                                    wsto[:, j * 1024 + kb * 128: j * 1024 + kb * 128 + 128],
                                    src[:, kb * T + tb * 512: kb * T + tb * 512 + 512],
                                    start=(kb == 0), stop=(kb == HB - 1))
                            pf.append(pfj)
                        g_s = WC.tile([128, 512], F32, tag="g_s")
                        nc.scalar.activation(g_s[:], pg[:], AF.Sigmoid)
                        f1_s = WC.tile([128, 512], F32, tag="f1_s")
                        nc.scalar.copy(f1_s[:], pf[0][:])
                        d_s = WC.tile([128, 512], F32, tag="d_s")
                        nc.vector.tensor_sub(d_s[:], pf[1][:], f1_s[:])
                        f13 = WC.tile([128, 512], F32, tag="f13")
                        nc.vector.tensor_add(f13[:], pf[2][:], f1_s[:])
                        t1 = WC.tile([128, 512], F32, tag="t1")
                        nc.vector.tensor_mul(t1[:], d_s[:], g_s[:])
                        t2 = WC.tile([128, 512], F32, tag="t2")
                        nc.vector.tensor_add(t2[:], t1[:], f13[:])
                        fo = WC.tile([128, 512], F32, tag="fo")
                        nc.scalar.activation(fo[:], t2[:], AF.Tanh)
                        nc.sync.dma_start(
                            d["fusT"].ap()[ht * 128:(ht + 1) * 128,
                                           tb * 512:(tb + 1) * 512], fo[:])

    nc.compile()
    return nc


def _get_nc(reps=1, bc="f16"):
    key = (reps, bc)
    if key not in _CACHE:
        _CACHE[key] = _build(reps, bc)
    return _CACHE[key]


def _prep_weight(w_t, cb, ob):
    """Pack W' [C,O] f16 so the per-output-tile lhsT DMA is contiguous.

    result[ot*128+p, kb*128+oo] = W'[kb*128+p, ot*128+oo]
    """
    return np.ascontiguousarray(
        w_t.reshape(cb, 128, ob, 128).transpose(2, 1, 0, 3).reshape(ob * 128, cb * 128))


def _hi_lo(x):
    hi = x.astype(ml_dtypes.bfloat16)
    lo = (x - hi.astype(np.float32)).astype(ml_dtypes.bfloat16)
    return hi, lo


def kernel(output, sent, template, W_gate, W_sent, W_template, W_output,
           _reps=None, _trace=False):
    from concourse.bass_utils import run_bass_kernel_spmd

    reps = _reps if _reps is not None else int(os.environ.get("BENCH_REPS", "1"))
    bc = os.environ.get("BENCH_BC", "f16")
    nc = _get_nc(reps, bc)

    f16 = np.float16 if bc == "f16" else ml_dtypes.bfloat16
    wg_p = _prep_weight(np.ascontiguousarray(W_gate.T).astype(f16), 24, 8)
    ws_p = _prep_weight(np.ascontiguousarray(W_sent.T).astype(f16), 8, 8)
    wt_p = _prep_weight(np.ascontiguousarray(W_template.T).astype(f16), 8, 8)
    wo_p = _prep_weight(np.ascontiguousarray(W_output.T).astype(f16), 8, 8)
    ident = np.eye(128, dtype=f16)

    in_maps = []
    for b in range(B):
        xt = np.ascontiguousarray(output[b].T)
        e1t = np.ascontiguousarray(sent[b].T)
        e2t = np.ascontiguousarray(template[b].T)
        xt_hi, xt_lo = _hi_lo(xt)
        e1t_hi, e1t_lo = _hi_lo(e1t)
        e2t_hi, e2t_lo = _hi_lo(e2t)
        in_maps.append({
            "xt_hi": xt_hi, "xt_lo": xt_lo,
            "e1t_hi": e1t_hi, "e1t_lo": e1t_lo,
            "e2t_hi": e2t_hi, "e2t_lo": e2t_lo,
            "e1f": sent[b].astype(f16), "e2f": template[b].astype(f16),
            "xtf": xt.astype(f16),
            "wg": wg_p, "ws": ws_p, "wt": wt_p, "wo": wo_p,
            "ident": ident,
        })

    res = run_bass_kernel_spmd(nc, in_maps, list(range(B)), trace=_trace)
    kernel.last_results = res

    fusion = np.stack([np.ascontiguousarray(res.results[b]["fusT"].T)
                       for b in range(B)])
    sent_weight = np.stack([res.results[b]["aw1"] for b in range(B)])
    template_weight = np.stack([res.results[b]["aw2"] for b in range(B)])
    return fusion, sent_weight, template_weight


# revision 41
# speedup vs baseline: 1.0350x; 1.0350x over previous
"""TRN2 Bass kernel for the two-encoder attention module.

Per batch element b (8 of them, one per NeuronCore):
    P1 = X @ E1^T          (T,S)   attention logits vs `sent`
    A1 = softmax(P1)               -> output sent_weight
    C1 = A1 @ E1           (T,H)
    P2/A2/C2 vs `template` (St)
    gate = sigmoid(cat(C1, C2, X) @ W_gate^T)        (T,H)
    fusion = tanh((1-gate)*C1@Ws^T + gate*C2@Wt^T + X@Wo^T)

Everything on-device is computed in a feature-major ("transposed") layout so
that every matmul contraction dim lands on SBUF partitions with no on-device
input transposes:
    - logits: lhsT = X^T (hi/lo bf16 split, 3-pass for fp32-grade accuracy),
      rhs = E^T (hi/lo).  Softmax along the free dim.
    - A^T obtained with PE transpose-mode matmuls (f16).
    - C^T  = E^T(natural lhsT) @ A^T    [f16]
    - gate^T, F^T via host-pretransposed/prepacked f16 weights.
Host side handles batch sharding across 8 cores, input transposes/casts and
the final un-transpose of fusion.
"""
import contextlib
import os

import ml_dtypes
import numpy as np

B, T, S, St, H = 8, 1024, 1024, 512, 1024
TB = T // 128           # 8 t-tiles
HB = H // 128           # 8 h-blocks
SB = S // 128           # 8 s-blocks (sent)
S2B = St // 128         # 4 s-blocks (template)
NT = T // 512           # 2 t-halves for 512-wide rhs

_CACHE = {}


def _build(reps=1, bc="f16"):
    import concourse.bacc as bacc
    import concourse.mybir as mybir
    import concourse.tile as tile

    dt = mybir.dt
    F32, F16, BF16 = dt.float32, dt.float16, dt.bfloat16
    if bc == "bf16":
        F16 = BF16  # dtype for the value/gate/fusion stages
    AF = mybir.ActivationFunctionType
    ALU = mybir.AluOpType
    AX = mybir.AxisListType

    nc = bacc.Bacc("TRN2", target_bir_lowering=False, debug=False, num_devices=8)

    d = {}
    for nm, shape, ddt in [
        ("xt_hi", [H, T], BF16), ("xt_lo", [H, T], BF16),
        ("e1t_hi", [H, S], BF16), ("e1t_lo", [H, S], BF16),
        ("e2t_hi", [H, St], BF16), ("e2t_lo", [H, St], BF16),
        ("e1f", [S, H], F16), ("e2f", [St, H], F16), ("xtf", [H, T], F16),
        ("wg", [H, 3 * H], F16), ("ws", [H, H], F16),
        ("wt", [H, H], F16), ("wo", [H, H], F16),
        ("ident", [128, 128], F16),
    ]:
        d[nm] = nc.dram_tensor(nm, shape, ddt, kind="ExternalInput")
    d["aw1"] = nc.dram_tensor("aw1", [T, S], F32, kind="ExternalOutput")
    d["aw2"] = nc.dram_tensor("aw2", [T, St], F32, kind="ExternalOutput")
    d["fusT"] = nc.dram_tensor("fusT", [H, T], F32, kind="ExternalOutput")

    with tile.TileContext(nc) as tc, contextlib.ExitStack() as ctx:
        P = ctx.enter_context(tc.tile_pool(name="persist", bufs=1))

        ident_t = P.tile([128, 128], F16, tag="ident")
        nc.gpsimd.dma_start(ident_t[:], d["ident"].ap()[:, :])

        def load_blocked(tag, dram, nblk, width, ddt, eng=None, nsplit=1):
            t = P.tile([128, nblk * width], ddt, tag=tag)
            eng = eng or nc.sync
            cw = width // nsplit
            for sp in range(nsplit):
                for b_ in range(nblk):
                    eng.dma_start(
                        t[:, b_ * width + sp * cw: b_ * width + (sp + 1) * cw],
                        dram.ap()[b_ * 128:(b_ + 1) * 128, sp * cw:(sp + 1) * cw])
            return t

        pending = []  # (a16 tile, it, nsb, a_t) transposes deferred one block

        def attention(it_range, nsb, xt_hi_t, xt_lo_t, et_hi_t, et_lo_t, a_t,
                      aw_dram, PSA, PST, WRK, STT, width):
            """One encoder's logits+softmax+transpose. width = S or St.

            Transposes for tile `it` are emitted after the logits of `it+1`
            so the softmax chain (DVE/ACT) has a full logits-block of slack
            before the PE reaches the transpose instructions.
            """
            nhalf = width // 512

            def flush_pending():
                for a16p, itp, nsbp, a_tp in pending:
                    for sb in range(nsbp):
                        pt = PST.tile([128, 128], F16, tag="tr")
                        nc.tensor.transpose(
                            pt[:], a16p[:, sb * 128:(sb + 1) * 128], ident_t[:])
                        nc.vector.tensor_copy(
                            a_tp[:, sb * T + itp * 128: sb * T + itp * 128 + 128],
                            pt[:])
                pending.clear()

            for it in it_range:
                # s-halves interleaved so consecutive matmuls share lhsT
                ps = [PSA.tile([128, 512], F32, tag="p", name=f"p{sh}")
                      for sh in range(nhalf)]
                i = 0
                for lt, rt in ((xt_hi_t, et_hi_t), (xt_hi_t, et_lo_t),
                               (xt_lo_t, et_hi_t)):
                    for hb in range(HB):
                        lhsT = lt[:, hb * T + it * 128: hb * T + it * 128 + 128]
                        for sh in range(nhalf):
                            nc.tensor.matmul(
                                ps[sh][:],
                                lhsT,
                                rt[:, hb * width + sh * 512: hb * width + sh * 512 + 512],
                                start=(i < nhalf), stop=(i >= (3 * HB - 1) * nhalf))
                            i += 1
                flush_pending()
                nm = STT.tile([128, 1], F32, tag="nm")
                if nhalf == 1:
                    nc.vector.tensor_reduce(nm[:], ps[0][:], axis=AX.X,
                                            op=ALU.max, negate=True)
                else:
                    nm0 = STT.tile([128, 1], F32, tag="nm0")
                    nm1 = STT.tile([128, 1], F32, tag="nm1")
                    nc.vector.tensor_reduce(nm0[:], ps[0][:], axis=AX.X,
                                            op=ALU.max, negate=True)
                    nc.vector.tensor_reduce(nm1[:], ps[1][:], axis=AX.X,
                                            op=ALU.max, negate=True)
                    nc.vector.tensor_tensor(nm[:], nm0[:], nm1[:], op=ALU.min)
                a_f32 = WRK.tile([128, width], F32, tag="a_f32")
                ssum = STT.tile([128, 1], F32, tag="ssum")
                if nhalf == 1:
                    nc.scalar.activation(a_f32[:], ps[0][:], AF.Exp,
                                         bias=nm[:], scale=1.0, accum_out=ssum[:])
                else:
                    s0 = STT.tile([128, 1], F32, tag="s0")
                    s1 = STT.tile([128, 1], F32, tag="s1")
                    nc.scalar.activation(a_f32[:, 0:512], ps[0][:], AF.Exp,
                                         bias=nm[:], scale=1.0, accum_out=s0[:])
                    nc.scalar.activation(a_f32[:, 512:1024], ps[1][:], AF.Exp,
                                         bias=nm[:], scale=1.0, accum_out=s1[:])
                    nc.vector.tensor_add(ssum[:], s0[:], s1[:])
                rinv = STT.tile([128, 1], F32, tag="rinv")
                nc.vector.reciprocal(rinv[:], ssum[:])
                an = WRK.tile([128, width], F32, tag="an")
                nc.scalar.activation(an[:], a_f32[:], AF.Copy, scale=rinv[:])
                nc.sync.dma_start(aw_dram.ap()[it * 128:(it + 1) * 128, :], an[:])
                a16 = WRK.tile([128, width], F16, tag="a16")
                nc.vector.tensor_scalar_mul(a16[:], a_f32[:], rinv[:])
                pending.append((a16, it, nsb, a_t))
            return flush_pending

        def ctx_matmul(c_t, ef_t, a_t, nsb, PSA):
            """C^T[k,t] = sum_s E[s,k] * A^T[s,t]. t-halves share lhsT."""
            for kt in range(HB):
                pcs = [PSA.tile([128, 512], F32, tag="p", name=f"pc{tb}")
                       for tb in range(NT)]
                for sb in range(nsb):
                    lhsT = ef_t[:, sb * H + kt * 128: sb * H + kt * 128 + 128]
                    for tb in range(NT):
                        nc.tensor.matmul(
                            pcs[tb][:], lhsT,
                            a_t[:, sb * T + tb * 512: sb * T + tb * 512 + 512],
                            start=(sb == 0), stop=(sb == nsb - 1))
                for tb in range(NT):
                    nc.scalar.copy(
                        c_t[:, kt * T + tb * 512: kt * T + tb * 512 + 512],
                        pcs[tb][:])

        for _rep in range(reps):
            pending.clear()
            # critical-path loads first: t-tile 0 logits need all of xt_hi and
            # e1t_hi; xt_lo/e1t_lo are needed one matmul-group later.
            xt_hi_t = load_blocked("xt_hi", d["xt_hi"], HB, T, BF16, nc.sync)
            e1t_hi_t = load_blocked("e1t_hi", d["e1t_hi"], HB, S, BF16, nc.sync)
            xt_lo_t = load_blocked("xt_lo", d["xt_lo"], HB, T, BF16, nc.sync)
            e1t_lo_t = load_blocked("e1t_lo", d["e1t_lo"], HB, S, BF16, nc.sync)
            a1t_t = P.tile([128, SB * T], F16, tag="a1t")
            a2t_t = P.tile([128, S2B * T], F16, tag="a2t")

            with tc.tile_pool(name="psA", bufs=6, space="PSUM") as PSA, \
                 tc.tile_pool(name="psT", bufs=2, space="PSUM") as PST, \
                 tc.tile_pool(name="wrkA", bufs=2) as WRK, \
                 tc.tile_pool(name="stats", bufs=3) as STT:
                # sent attention, then template attention (fills PE while the
                # tail of sent softmax/transposes completes), then both C^T.
                attention(range(TB), SB, xt_hi_t, xt_lo_t, e1t_hi_t, e1t_lo_t,
                          a1t_t, d["aw1"], PSA, PST, WRK, STT, S)
                e2t_hi_t = load_blocked("e2t_hi", d["e2t_hi"], HB, St, BF16,
                                        nc.gpsimd)
                e2t_lo_t = load_blocked("e2t_lo", d["e2t_lo"], HB, St, BF16,
                                        nc.gpsimd)
                e1f_t = load_blocked("e1f", d["e1f"], SB, H, F16, nc.gpsimd)
                e2f_t = load_blocked("e2f", d["e2f"], S2B, H, F16, nc.gpsimd)
                flush_last = attention(
                    range(TB), S2B, xt_hi_t, xt_lo_t, e2t_hi_t, e2t_lo_t,
                    a2t_t, d["aw2"], PSA, PST, WRK, STT, St)
                flush_last()
                c1t_t = P.tile([128, HB * T], F16, tag="e1t_hi")  # slot reuse
                ctx_matmul(c1t_t, e1f_t, a1t_t, SB, PSA)
                c2t_t = P.tile([128, HB * T], F16, tag="e1t_lo")  # slot reuse
                ctx_matmul(c2t_t, e2f_t, a2t_t, S2B, PSA)

            # fused gate + fusion stage (all f16 matmuls)
            xtf_t = load_blocked("xt_hi", d["xtf"], HB, T, F16)  # slot reuse
            with tc.tile_pool(name="psC", bufs=2, space="PSUM") as PSC, \
                 tc.tile_pool(name="wrkC", bufs=2) as WC:
                for ht in range(HB):
                    wgt = WC.tile([128, 24 * 128], F16, tag="wg")
                    nc.sync.dma_start(wgt[:], d["wg"].ap()[ht * 128:(ht + 1) * 128, :])
                    wsto = WC.tile([128, 3 * 8 * 128], F16, tag="wsto")
                    for j, wnm in enumerate(("ws", "wt", "wo")):
                        nc.sync.dma_start(
                            wsto[:, j * 1024:(j + 1) * 1024],
                            d[wnm].ap()[ht * 128:(ht + 1) * 128, :])
                    for tb in range(NT):
                        pg = PSC.tile([128, 512], F32, tag="g")
                        i = 0
                        for src in (c1t_t, c2t_t, xtf_t):
                            for kb in range(HB):
                                nc.tensor.matmul(
                                    pg[:], wgt[:, i * 128:(i + 1) * 128],
                                    src[:, kb * T + tb * 512: kb * T + tb * 512 + 512],
                                    start=(i == 0), stop=(i == 23))
                                i += 1
                        pf = []
                        for j, src in enumerate((c1t_t, c2t_t, xtf_t)):
                            pfj = PSC.tile([128, 512], F32, tag=f"f{j}")
                            for kb in range(HB):
                                nc.tensor.matmul(
                                    pfj[:],
                                    wsto[:, j * 1024 + kb * 128: j * 1024 + kb * 128 + 128],
                                    src[:, kb * T + tb * 512: kb * T + tb * 512 + 512],
                                    start=(kb == 0), stop=(kb == HB - 1))
                            pf.append(pfj)
                        g_s = WC.tile([128, 512], F32, tag="g_s")
                        nc.scalar.activation(g_s[:], pg[:], AF.Sigmoid)
                        f1_s = WC.tile([128, 512], F32, tag="f1_s")
                        nc.scalar.copy(f1_s[:], pf[0][:])
                        d_s = WC.tile([128, 512], F32, tag="d_s")
                        nc.vector.tensor_sub(d_s[:], pf[1][:], f1_s[:])
                        f13 = WC.tile([128, 512], F32, tag="f13")
                        nc.vector.tensor_add(f13[:], pf[2][:], f1_s[:])
                        t1 = WC.tile([128, 512], F32, tag="t1")
                        nc.vector.tensor_mul(t1[:], d_s[:], g_s[:])
                        t2 = WC.tile([128, 512], F32, tag="t2")
                        nc.vector.tensor_add(t2[:], t1[:], f13[:])
                        fo = WC.tile([128, 512], F32, tag="fo")
                        nc.scalar.activation(fo[:], t2[:], AF.Tanh)
                        nc.sync.dma_start(
                            d["fusT"].ap()[ht * 128:(ht + 1) * 128,
                                           tb * 512:(tb + 1) * 512], fo[:])

    nc.compile()
    return nc


def _get_nc(reps=1, bc="f16"):
    key = (reps, bc)
    if key not in _CACHE:
        _CACHE[key] = _build(reps, bc)
    return _CACHE[key]


def _prep_weight(w_t, cb, ob):
    """Pack W' [C,O] f16 so the per-output-tile lhsT DMA is contiguous.

    result[ot*128+p, kb*128+oo] = W'[kb*128+p, ot*128+oo]
    """
    return np.ascontiguousarray(
        w_t.reshape(cb, 128, ob, 128).transpose(2, 1, 0, 3).reshape(ob * 128, cb * 128))


def _hi_lo(x):
    hi = x.astype(ml_dtypes.bfloat16)
    lo = (x - hi.astype(np.float32)).astype(ml_dtypes.bfloat16)
    return hi, lo


def kernel(output, sent, template, W_gate, W_sent, W_template, W_output,
           _reps=None, _trace=False):
    from concourse.bass_utils import run_bass_kernel_spmd

    reps = _reps if _reps is not None else int(os.environ.get("BENCH_REPS", "1"))
    bc = os.environ.get("BENCH_BC", "f16")
    nc = _get_nc(reps, bc)

    f16 = np.float16 if bc == "f16" else ml_dtypes.bfloat16
    wg_p = _prep_weight(np.ascontiguousarray(W_gate.T).astype(f16), 24, 8)
    ws_p = _prep_weight(np.ascontiguousarray(W_sent.T).astype(f16), 8, 8)
    wt_p = _prep_weight(np.ascontiguousarray(W_template.T).astype(f16), 8, 8)
    wo_p = _prep_weight(np.ascontiguousarray(W_output.T).astype(f16), 8, 8)
    ident = np.eye(128, dtype=f16)

    in_maps = []
    for b in range(B):
        xt = np.ascontiguousarray(output[b].T)
        e1t = np.ascontiguousarray(sent[b].T)
        e2t = np.ascontiguousarray(template[b].T)
        xt_hi, xt_lo = _hi_lo(xt)
        e1t_hi, e1t_lo = _hi_lo(e1t)
        e2t_hi, e2t_lo = _hi_lo(e2t)
        in_maps.append({
            "xt_hi": xt_hi, "xt_lo": xt_lo,
            "e1t_hi": e1t_hi, "e1t_lo": e1t_lo,
            "e2t_hi": e2t_hi, "e2t_lo": e2t_lo,
            "e1f": sent[b].astype(f16), "e2f": template[b].astype(f16),
            "xtf": xt.astype(f16),
            "wg": wg_p, "ws": ws_p, "wt": wt_p, "wo": wo_p,
            "ident": ident,
        })

    res = run_bass_kernel_spmd(nc, in_maps, list(range(B)), trace=_trace)
    kernel.last_results = res

    fusion = np.stack([np.ascontiguousarray(res.results[b]["fusT"].T)
                       for b in range(B)])
    sent_weight = np.stack([res.results[b]["aw1"] for b in range(B)])
    template_weight = np.stack([res.results[b]["aw2"] for b in range(B)])
    return fusion, sent_weight, template_weight


# revision 45
# speedup vs baseline: 1.0355x; 1.0005x over previous
"""TRN2 Bass kernel for the two-encoder attention module.

Per batch element b (8 of them, one per NeuronCore):
    P1 = X @ E1^T          (T,S)   attention logits vs `sent`
    A1 = softmax(P1)               -> output sent_weight
    C1 = A1 @ E1           (T,H)
    P2/A2/C2 vs `template` (St)
    gate = sigmoid(cat(C1, C2, X) @ W_gate^T)        (T,H)
    fusion = tanh((1-gate)*C1@Ws^T + gate*C2@Wt^T + X@Wo^T)

Everything on-device is computed in a feature-major ("transposed") layout so
that every matmul contraction dim lands on SBUF partitions with no on-device
input transposes:
    - logits: lhsT = X^T (hi/lo bf16 split, 3-pass for fp32-grade accuracy),
      rhs = E^T (hi/lo).  Softmax along the free dim.
    - A^T obtained with PE transpose-mode matmuls (f16).
    - C^T  = E^T(natural lhsT) @ A^T    [f16]
    - gate^T, F^T via host-pretransposed/prepacked f16 weights.
Host side handles batch sharding across 8 cores, input transposes/casts and
the final un-transpose of fusion.
"""
import contextlib
import os

import ml_dtypes
import numpy as np

B, T, S, St, H = 8, 1024, 1024, 512, 1024
TB = T // 128           # 8 t-tiles
HB = H // 128           # 8 h-blocks
SB = S // 128           # 8 s-blocks (sent)
S2B = St // 128         # 4 s-blocks (template)
NT = T // 512           # 2 t-halves for 512-wide rhs

_CACHE = {}


def _build(reps=1, bc="f16"):
    import concourse.bacc as bacc
    import concourse.mybir as mybir
    import concourse.tile as tile

    dt = mybir.dt
    F32, F16, BF16 = dt.float32, dt.float16, dt.bfloat16
    if bc == "bf16":
        F16 = BF16  # dtype for the value/gate/fusion stages
    AF = mybir.ActivationFunctionType
    ALU = mybir.AluOpType
    AX = mybir.AxisListType

    nc = bacc.Bacc("TRN2", target_bir_lowering=False, debug=False, num_devices=8)

    d = {}
    for nm, shape, ddt in [
        ("xt_hi", [H, T], BF16), ("xt_lo", [H, T], BF16),
        ("e1t_hi", [H, S], BF16), ("e1t_lo", [H, S], BF16),
        ("e2t_hi", [H, St], BF16), ("e2t_lo", [H, St], BF16),
        ("e1f", [S, H], F16), ("e2f", [St, H], F16), ("xtf", [H, T], F16),
        ("wg", [H, 3 * H], F16), ("ws", [H, H], F16),
        ("wt", [H, H], F16), ("wo", [H, H], F16),
        ("ident", [128, 128], F16),
    ]:
        d[nm] = nc.dram_tensor(nm, shape, ddt, kind="ExternalInput")
    d["aw1"] = nc.dram_tensor("aw1", [T, S], F32, kind="ExternalOutput")
    d["aw2"] = nc.dram_tensor("aw2", [T, St], F32, kind="ExternalOutput")
    d["fusT"] = nc.dram_tensor("fusT", [H, T], F32, kind="ExternalOutput")

    with tile.TileContext(nc) as tc, contextlib.ExitStack() as ctx:
        P = ctx.enter_context(tc.tile_pool(name="persist", bufs=1))

        ident_t = P.tile([128, 128], F16, tag="ident")
        nc.gpsimd.dma_start(ident_t[:], d["ident"].ap()[:, :])

        def load_blocked(tag, dram, nblk, width, ddt, eng=None, nsplit=1):
            t = P.tile([128, nblk * width], ddt, tag=tag)
            eng = eng or nc.sync
            cw = width // nsplit
            for sp in range(nsplit):
                for b_ in range(nblk):
                    eng.dma_start(
                        t[:, b_ * width + sp * cw: b_ * width + (sp + 1) * cw],
                        dram.ap()[b_ * 128:(b_ + 1) * 128, sp * cw:(sp + 1) * cw])
            return t

        pending = []  # (a16 tile, it, nsb, a_t) transposes deferred one block

        def attention(it_range, nsb, xt_hi_t, xt_lo_t, et_hi_t, et_lo_t, a_t,
                      aw_dram, PSA, PST, WRK, STT, width):
            """One encoder's logits+softmax+transpose. width = S or St.

            Transposes for tile `it` are emitted after the logits of `it+1`
            so the softmax chain (DVE/ACT) has a full logits-block of slack
            before the PE reaches the transpose instructions.
            """
            nhalf = width // 512

            def flush_pending():
                for a16p, itp, nsbp, a_tp in pending:
                    for sb in range(nsbp):
                        pt = PST.tile([128, 128], F16, tag="tr")
                        nc.tensor.transpose(
                            pt[:], a16p[:, sb * 128:(sb + 1) * 128], ident_t[:])
                        nc.vector.tensor_copy(
                            a_tp[:, sb * T + itp * 128: sb * T + itp * 128 + 128],
                            pt[:])
                pending.clear()

            # Tiles processed in pairs, pass-major: both tiles' (hi,hi) pass
            # runs before any (lo,*) pass, so the kernel head only waits on
            # the hi tensors' DMA; pass order matches the DMA issue order.
            passes = ((xt_hi_t, et_hi_t), (xt_lo_t, et_hi_t), (xt_hi_t, et_lo_t))
            it_list = list(it_range)
            for it0 in range(0, len(it_list), 2):
                pair = it_list[it0:it0 + 2]
                ps = {it: [PSA.tile([128, 512], F32, tag="p", name=f"p{it % 2}{sh}")
                           for sh in range(nhalf)] for it in pair}
                for pi, (lt, rt) in enumerate(passes):
                    for it in pair:
                        for hb in range(HB):
                            lhsT = lt[:, hb * T + it * 128: hb * T + it * 128 + 128]
                            for sh in range(nhalf):
                                nc.tensor.matmul(
                                    ps[it][sh][:],
                                    lhsT,
                                    rt[:, hb * width + sh * 512:
                                       hb * width + sh * 512 + 512],
                                    start=(pi == 0 and hb == 0),
                                    stop=(pi == 2 and hb == HB - 1))
                flush_pending()
                for it in pair:
                    softmax_tile(it, ps[it], nsb, a_t, aw_dram, WRK, STT, width,
                                 nhalf)
            return flush_pending

        def softmax_tile(it, ps, nsb, a_t, aw_dram, WRK, STT, width, nhalf):
                nm = STT.tile([128, 1], F32, tag="nm")
                if nhalf == 1:
                    nc.vector.tensor_reduce(nm[:], ps[0][:], axis=AX.X,
                                            op=ALU.max, negate=True)
                else:
                    nm0 = STT.tile([128, 1], F32, tag="nm0")
                    nm1 = STT.tile([128, 1], F32, tag="nm1")
                    nc.vector.tensor_reduce(nm0[:], ps[0][:], axis=AX.X,
                                            op=ALU.max, negate=True)
                    nc.vector.tensor_reduce(nm1[:], ps[1][:], axis=AX.X,
                                            op=ALU.max, negate=True)
                    nc.vector.tensor_tensor(nm[:], nm0[:], nm1[:], op=ALU.min)
                a_f32 = WRK.tile([128, width], F32, tag="a_f32")
                ssum = STT.tile([128, 1], F32, tag="ssum")
                if nhalf == 1:
                    nc.scalar.activation(a_f32[:], ps[0][:], AF.Exp,
                                         bias=nm[:], scale=1.0, accum_out=ssum[:])
                else:
                    s0 = STT.tile([128, 1], F32, tag="s0")
                    s1 = STT.tile([128, 1], F32, tag="s1")
                    nc.scalar.activation(a_f32[:, 0:512], ps[0][:], AF.Exp,
                                         bias=nm[:], scale=1.0, accum_out=s0[:])
                    nc.scalar.activation(a_f32[:, 512:1024], ps[1][:], AF.Exp,
                                         bias=nm[:], scale=1.0, accum_out=s1[:])
                    nc.vector.tensor_add(ssum[:], s0[:], s1[:])
                rinv = STT.tile([128, 1], F32, tag="rinv")
                nc.vector.reciprocal(rinv[:], ssum[:])
                an = WRK.tile([128, width], F32, tag="an")
                nc.scalar.activation(an[:], a_f32[:], AF.Copy, scale=rinv[:])
                nc.sync.dma_start(aw_dram.ap()[it * 128:(it + 1) * 128, :], an[:])
                a16 = WRK.tile([128, width], F16, tag="a16", bufs=4)
                nc.vector.tensor_scalar_mul(a16[:], a_f32[:], rinv[:])
                pending.append((a16, it, nsb, a_t))

        def ctx_matmul(c_t, ef_t, a_t, nsb, PSA):
            """C^T[k,t] = sum_s E[s,k] * A^T[s,t]. t-halves share lhsT."""
            for kt in range(HB):
                pcs = [PSA.tile([128, 512], F32, tag="p", name=f"pc{tb}")
                       for tb in range(NT)]
                for sb in range(nsb):
                    lhsT = ef_t[:, sb * H + kt * 128: sb * H + kt * 128 + 128]
                    for tb in range(NT):
                        nc.tensor.matmul(
                            pcs[tb][:], lhsT,
                            a_t[:, sb * T + tb * 512: sb * T + tb * 512 + 512],
                            start=(sb == 0), stop=(sb == nsb - 1))
                for tb in range(NT):
                    nc.scalar.copy(
                        c_t[:, kt * T + tb * 512: kt * T + tb * 512 + 512],
                        pcs[tb][:])

        for _rep in range(reps):
            pending.clear()
            # critical-path loads first: t-tile 0 logits need all of xt_hi and
            # e1t_hi; xt_lo/e1t_lo are needed one matmul-group later.
            xt_hi_t = load_blocked("xt_hi", d["xt_hi"], HB, T, BF16, nc.sync)
            e1t_hi_t = load_blocked("e1t_hi", d["e1t_hi"], HB, S, BF16, nc.sync)
            xt_lo_t = load_blocked("xt_lo", d["xt_lo"], HB, T, BF16, nc.sync)
            e1t_lo_t = load_blocked("e1t_lo", d["e1t_lo"], HB, S, BF16, nc.sync)
            a1t_t = P.tile([128, SB * T], F16, tag="a1t")
            a2t_t = P.tile([128, S2B * T], F16, tag="a2t")

            with tc.tile_pool(name="psA", bufs=6, space="PSUM") as PSA, \
                 tc.tile_pool(name="psT", bufs=2, space="PSUM") as PST, \
                 tc.tile_pool(name="wrkA", bufs=2) as WRK, \
                 tc.tile_pool(name="stats", bufs=4) as STT:
                # sent attention, then template attention (fills PE while the
                # tail of sent softmax/transposes completes), then both C^T.
                attention(range(TB), SB, xt_hi_t, xt_lo_t, e1t_hi_t, e1t_lo_t,
                          a1t_t, d["aw1"], PSA, PST, WRK, STT, S)
                e2t_hi_t = load_blocked("e2t_hi", d["e2t_hi"], HB, St, BF16,
                                        nc.gpsimd)
                e2t_lo_t = load_blocked("e2t_lo", d["e2t_lo"], HB, St, BF16,
                                        nc.gpsimd)
                e1f_t = load_blocked("e1f", d["e1f"], SB, H, F16, nc.gpsimd)
                e2f_t = load_blocked("e2f", d["e2f"], S2B, H, F16, nc.gpsimd)
                flush_last = attention(
                    range(TB), S2B, xt_hi_t, xt_lo_t, e2t_hi_t, e2t_lo_t,
                    a2t_t, d["aw2"], PSA, PST, WRK, STT, St)
                flush_last()
                c1t_t = P.tile([128, HB * T], F16, tag="e1t_hi")  # slot reuse
                ctx_matmul(c1t_t, e1f_t, a1t_t, SB, PSA)
                c2t_t = P.tile([128, HB * T], F16, tag="e1t_lo")  # slot reuse
                ctx_matmul(c2t_t, e2f_t, a2t_t, S2B, PSA)

            # fused gate + fusion stage (all f16 matmuls)
            xtf_t = load_blocked("xt_hi", d["xtf"], HB, T, F16)  # slot reuse
            with tc.tile_pool(name="psC", bufs=2, space="PSUM") as PSC, \
                 tc.tile_pool(name="wrkC", bufs=2) as WC:
                for ht in range(HB):
                    wgt = WC.tile([128, 24 * 128], F16, tag="wg")
                    nc.sync.dma_start(wgt[:], d["wg"].ap()[ht * 128:(ht + 1) * 128, :])
                    wsto = WC.tile([128, 3 * 8 * 128], F16, tag="wsto")
                    for j, wnm in enumerate(("ws", "wt", "wo")):
                        nc.sync.dma_start(
                            wsto[:, j * 1024:(j + 1) * 1024],
                            d[wnm].ap()[ht * 128:(ht + 1) * 128, :])
                    for tb in range(NT):
                        pg = PSC.tile([128, 512], F32, tag="g")
                        i = 0
                        for src in (c1t_t, c2t_t, xtf_t):
                            for kb in range(HB):
                                nc.tensor.matmul(
                                    pg[:], wgt[:, i * 128:(i + 1) * 128],
                                    src[:, kb * T + tb * 512: kb * T + tb * 512 + 512],
                                    start=(i == 0), stop=(i == 23))
                                i += 1
                        pf = []
                        for j, src in enumerate((c1t_t, c2t_t, xtf_t)):
                            pfj = PSC.tile([128, 512], F32, tag=f"f{j}")
                            for kb in range(HB):
                                nc.tensor.matmul(
                                    pfj[:],
                                    wsto[:, j * 1024 + kb * 128: j * 1024 + kb * 128 + 128],
                                    src[:, kb * T + tb * 512: kb * T + tb * 512 + 512],
                                    start=(kb == 0), stop=(kb == HB - 1))
                            pf.append(pfj)
                        g_s = WC.tile([128, 512], F32, tag="g_s")
                        nc.scalar.activation(g_s[:], pg[:], AF.Sigmoid)
                        f1_s = WC.tile([128, 512], F32, tag="f1_s")
                        nc.scalar.copy(f1_s[:], pf[0][:])
                        d_s = WC.tile([128, 512], F32, tag="d_s")
                        nc.vector.tensor_sub(d_s[:], pf[1][:], f1_s[:])
                        f13 = WC.tile([128, 512], F32, tag="f13")
                        nc.vector.tensor_add(f13[:], pf[2][:], f1_s[:])
                        t1 = WC.tile([128, 512], F32, tag="t1")
                        nc.vector.tensor_mul(t1[:], d_s[:], g_s[:])
                        t2 = WC.tile([128, 512], F32, tag="t2")
                        nc.vector.tensor_add(t2[:], t1[:], f13[:])
                        fo = WC.tile([128, 512], F32, tag="fo")
                        nc.scalar.activation(fo[:], t2[:], AF.Tanh)
                        nc.sync.dma_start(
                            d["fusT"].ap()[ht * 128:(ht + 1) * 128,
                                           tb * 512:(tb + 1) * 512], fo[:])

    nc.compile()
    return nc


def _get_nc(reps=1, bc="f16"):
    key = (reps, bc)
    if key not in _CACHE:
        _CACHE[key] = _build(reps, bc)
    return _CACHE[key]


def _prep_weight(w_t, cb, ob):
    """Pack W' [C,O] f16 so the per-output-tile lhsT DMA is contiguous.

    result[ot*128+p, kb*128+oo] = W'[kb*128+p, ot*128+oo]
    """
    return np.ascontiguousarray(
        w_t.reshape(cb, 128, ob, 128).transpose(2, 1, 0, 3).reshape(ob * 128, cb * 128))


def _hi_lo(x):
    hi = x.astype(ml_dtypes.bfloat16)
    lo = (x - hi.astype(np.float32)).astype(ml_dtypes.bfloat16)
    return hi, lo


def kernel(output, sent, template, W_gate, W_sent, W_template, W_output,
           _reps=None, _trace=False):
    from concourse.bass_utils import run_bass_kernel_spmd

    reps = _reps if _reps is not None else int(os.environ.get("BENCH_REPS", "1"))
    bc = os.environ.get("BENCH_BC", "f16")
    nc = _get_nc(reps, bc)

    f16 = np.float16 if bc == "f16" else ml_dtypes.bfloat16
    wg_p = _prep_weight(np.ascontiguousarray(W_gate.T).astype(f16), 24, 8)
    ws_p = _prep_weight(np.ascontiguousarray(W_sent.T).astype(f16), 8, 8)
    wt_p = _prep_weight(np.ascontiguousarray(W_template.T).astype(f16), 8, 8)
    wo_p = _prep_weight(np.ascontiguousarray(W_output.T).astype(f16), 8, 8)
    ident = np.eye(128, dtype=f16)

    in_maps = []
    for b in range(B):
        xt = np.ascontiguousarray(output[b].T)
        e1t = np.ascontiguousarray(sent[b].T)
        e2t = np.ascontiguousarray(template[b].T)
        xt_hi, xt_lo = _hi_lo(xt)
        e1t_hi, e1t_lo = _hi_lo(e1t)
        e2t_hi, e2t_lo = _hi_lo(e2t)
        in_maps.append({
            "xt_hi": xt_hi, "xt_lo": xt_lo,
            "e1t_hi": e1t_hi, "e1t_lo": e1t_lo,
            "e2t_hi": e2t_hi, "e2t_lo": e2t_lo,
            "e1f": sent[b].astype(f16), "e2f": template[b].astype(f16),
            "xtf": xt.astype(f16),
            "wg": wg_p, "ws": ws_p, "wt": wt_p, "wo": wo_p,
            "ident": ident,
        })

    res = run_bass_kernel_spmd(nc, in_maps, list(range(B)), trace=_trace)
    kernel.last_results = res

    fusion = np.stack([np.ascontiguousarray(res.results[b]["fusT"].T)
                       for b in range(B)])
    sent_weight = np.stack([res.results[b]["aw1"] for b in range(B)])
    template_weight = np.stack([res.results[b]["aw2"] for b in range(B)])
    return fusion, sent_weight, template_weight


# revision 47
# speedup vs baseline: 1.0772x; 1.0403x over previous
"""TRN2 Bass kernel for the two-encoder attention module.

Per batch element b (8 of them, one per NeuronCore):
    P1 = X @ E1^T          (T,S)   attention logits vs `sent`
    A1 = softmax(P1)               -> output sent_weight
    C1 = A1 @ E1           (T,H)
    P2/A2/C2 vs `template` (St)
    gate = sigmoid(cat(C1, C2, X) @ W_gate^T)        (T,H)
    fusion = tanh((1-gate)*C1@Ws^T + gate*C2@Wt^T + X@Wo^T)

Everything on-device is computed in a feature-major ("transposed") layout so
that every matmul contraction dim lands on SBUF partitions with no on-device
input transposes:
    - logits: lhsT = X^T (hi/lo bf16 split, 3-pass for fp32-grade accuracy),
      rhs = E^T (hi/lo).  Softmax along the free dim.
    - A^T obtained with PE transpose-mode matmuls (f16).
    - C^T  = E^T(natural lhsT) @ A^T    [f16]
    - gate^T, F^T via host-pretransposed/prepacked f16 weights.
Host side handles batch sharding across 8 cores, input transposes/casts and
the final un-transpose of fusion.
"""
import contextlib
import os

import ml_dtypes
import numpy as np

B, T, S, St, H = 8, 1024, 1024, 512, 1024
TB = T // 128           # 8 t-tiles
HB = H // 128           # 8 h-blocks
SB = S // 128           # 8 s-blocks (sent)
S2B = St // 128         # 4 s-blocks (template)
NT = T // 512           # 2 t-halves for 512-wide rhs

_CACHE = {}


def _build(reps=1, bc="f16"):
    import concourse.bacc as bacc
    import concourse.mybir as mybir
    import concourse.tile as tile

    dt = mybir.dt
    F32, F16, BF16 = dt.float32, dt.float16, dt.bfloat16
    if bc == "bf16":
        F16 = BF16  # dtype for the value/gate/fusion stages
    AF = mybir.ActivationFunctionType
    ALU = mybir.AluOpType
    AX = mybir.AxisListType

    nc = bacc.Bacc("TRN2", target_bir_lowering=False, debug=False, num_devices=8)

    d = {}
    for nm, shape, ddt in [
        ("xt_hi", [H, T], BF16), ("xt_lo", [H, T], BF16),
        ("e1t_hi", [H, S], BF16), ("e1t_lo", [H, S], BF16),
        ("e2t_hi", [H, St], BF16), ("e2t_lo", [H, St], BF16),
        ("e1f", [S, H], F16), ("e2f", [St, H], F16), ("xtf", [H, T], F16),
        ("wg", [H, 3 * H], F16), ("ws", [H, H], F16),
        ("wt", [H, H], F16), ("wo", [H, H], F16),
        ("ident", [128, 128], F16),
    ]:
        d[nm] = nc.dram_tensor(nm, shape, ddt, kind="ExternalInput")
    d["aw1"] = nc.dram_tensor("aw1", [T, S], F32, kind="ExternalOutput")
    d["aw2"] = nc.dram_tensor("aw2", [T, St], F32, kind="ExternalOutput")
    d["fusT"] = nc.dram_tensor("fusT", [H, T], F32, kind="ExternalOutput")

    with tile.TileContext(nc) as tc, contextlib.ExitStack() as ctx:
        P = ctx.enter_context(tc.tile_pool(name="persist", bufs=1))

        ident_t = P.tile([128, 128], F16, tag="ident")
        nc.gpsimd.dma_start(ident_t[:], d["ident"].ap()[:, :])

        def load_blocked(tag, dram, nblk, width, ddt, eng=None, nsplit=1):
            t = P.tile([128, nblk * width], ddt, tag=tag)
            eng = eng or nc.sync
            cw = width // nsplit
            for sp in range(nsplit):
                for b_ in range(nblk):
                    eng.dma_start(
                        t[:, b_ * width + sp * cw: b_ * width + (sp + 1) * cw],
                        dram.ap()[b_ * 128:(b_ + 1) * 128, sp * cw:(sp + 1) * cw])
            return t

        pending = []  # (a16 tile, it, nsb, a_t) transposes deferred one block

        def attention(it_range, nsb, xt_hi_t, xt_lo_t, et_hi_t, et_lo_t, a_t,
                      aw_dram, PSA, PST, WRK, STT, width):
            """One encoder's logits+softmax+transpose. width = S or St.

            Transposes for tile `it` are emitted after the logits of `it+1`
            so the softmax chain (DVE/ACT) has a full logits-block of slack
            before the PE reaches the transpose instructions.
            """
            nhalf = width // 512

            def flush_pending():
                for a16p, itp, nsbp, a_tp in pending:
                    for sb in range(nsbp):
                        pt = PST.tile([128, 128], F16, tag="tr")
                        nc.tensor.transpose(
                            pt[:], a16p[:, sb * 128:(sb + 1) * 128], ident_t[:])
                        nc.vector.tensor_copy(
                            a_tp[:, sb * T + itp * 128: sb * T + itp * 128 + 128],
                            pt[:])
                pending.clear()

            # Tiles processed in pairs, pass-major: both tiles' (hi,hi) pass
            # runs before any (lo,*) pass, so the kernel head only waits on
            # the hi tensors' DMA; pass order matches the DMA issue order.
            passes = ((xt_hi_t, et_hi_t), (xt_lo_t, et_hi_t), (xt_hi_t, et_lo_t))
            it_list = list(it_range)
            for it0 in range(0, len(it_list), 2):
                pair = it_list[it0:it0 + 2]
                ps = {it: [PSA.tile([128, 512], F32, tag="p", name=f"p{it % 2}{sh}")
                           for sh in range(nhalf)] for it in pair}
                for pi, (lt, rt) in enumerate(passes):
                    for it in pair:
                        for hb in range(HB):
                            lhsT = lt[:, hb * T + it * 128: hb * T + it * 128 + 128]
                            for sh in range(nhalf):
                                nc.tensor.matmul(
                                    ps[it][sh][:],
                                    lhsT,
                                    rt[:, hb * width + sh * 512:
                                       hb * width + sh * 512 + 512],
                                    start=(pi == 0 and hb == 0),
                                    stop=(pi == 2 and hb == HB - 1))
                flush_pending()
                for it in pair:
                    softmax_tile(it, ps[it], nsb, a_t, aw_dram, WRK, STT, width,
                                 nhalf)
            return flush_pending

        def softmax_tile(it, ps, nsb, a_t, aw_dram, WRK, STT, width, nhalf):
                nm = STT.tile([128, 1], F32, tag="nm")
                if nhalf == 1:
                    nc.vector.tensor_reduce(nm[:], ps[0][:], axis=AX.X,
                                            op=ALU.max, negate=True)
                else:
                    nm0 = STT.tile([128, 1], F32, tag="nm0")
                    nm1 = STT.tile([128, 1], F32, tag="nm1")
                    nc.vector.tensor_reduce(nm0[:], ps[0][:], axis=AX.X,
                                            op=ALU.max, negate=True)
                    nc.vector.tensor_reduce(nm1[:], ps[1][:], axis=AX.X,
                                            op=ALU.max, negate=True)
                    nc.vector.tensor_tensor(nm[:], nm0[:], nm1[:], op=ALU.min)
                a_f32 = WRK.tile([128, width], F32, tag="a_f32")
                ssum = STT.tile([128, 1], F32, tag="ssum")
                if nhalf == 1:
                    nc.scalar.activation(a_f32[:], ps[0][:], AF.Exp,
                                         bias=nm[:], scale=1.0, accum_out=ssum[:])
                else:
                    s0 = STT.tile([128, 1], F32, tag="s0")
                    s1 = STT.tile([128, 1], F32, tag="s1")
                    nc.scalar.activation(a_f32[:, 0:512], ps[0][:], AF.Exp,
                                         bias=nm[:], scale=1.0, accum_out=s0[:])
                    nc.scalar.activation(a_f32[:, 512:1024], ps[1][:], AF.Exp,
                                         bias=nm[:], scale=1.0, accum_out=s1[:])
                    nc.vector.tensor_add(ssum[:], s0[:], s1[:])
                rinv = STT.tile([128, 1], F32, tag="rinv")
                nc.vector.reciprocal(rinv[:], ssum[:])
                an = WRK.tile([128, width], F32, tag="an")
                nc.scalar.activation(an[:], a_f32[:], AF.Copy, scale=rinv[:])
                nc.sync.dma_start(aw_dram.ap()[it * 128:(it + 1) * 128, :], an[:])
                a16 = WRK.tile([128, width], F16, tag="a16", bufs=4)
                nc.vector.tensor_scalar_mul(a16[:], a_f32[:], rinv[:])
                pending.append((a16, it, nsb, a_t))

        def ctx_matmul(c_t, ef_t, a_t, nsb, PSA):
            """C^T[k,t] = sum_s E[s,k] * A^T[s,t]. t-halves share lhsT."""
            for kt in range(HB):
                pcs = [PSA.tile([128, 512], F32, tag="p", name=f"pc{tb}")
                       for tb in range(NT)]
                for sb in range(nsb):
                    lhsT = ef_t[:, sb * H + kt * 128: sb * H + kt * 128 + 128]
                    for tb in range(NT):
                        nc.tensor.matmul(
                            pcs[tb][:], lhsT,
                            a_t[:, sb * T + tb * 512: sb * T + tb * 512 + 512],
                            start=(sb == 0), stop=(sb == nsb - 1))
                for tb in range(NT):
                    nc.scalar.copy(
                        c_t[:, kt * T + tb * 512: kt * T + tb * 512 + 512],
                        pcs[tb][:])

        for _rep in range(reps):
            pending.clear()
            # critical-path loads first: t-tile 0 logits need all of xt_hi and
            # e1t_hi; xt_lo/e1t_lo are needed one matmul-group later.
            xt_hi_t = load_blocked("xt_hi", d["xt_hi"], HB, T, BF16, nc.sync)
            e1t_hi_t = load_blocked("e1t_hi", d["e1t_hi"], HB, S, BF16, nc.sync)
            xt_lo_t = load_blocked("xt_lo", d["xt_lo"], HB, T, BF16, nc.sync)
            e1t_lo_t = load_blocked("e1t_lo", d["e1t_lo"], HB, S, BF16, nc.sync)
            # non-critical loads behind the criticals on the same queue, so
            # they never compete with them for HBM bandwidth at the head
            e2t_hi_t = load_blocked("e2t_hi", d["e2t_hi"], HB, St, BF16, nc.sync)
            e2t_lo_t = load_blocked("e2t_lo", d["e2t_lo"], HB, St, BF16, nc.sync)
            e1f_t = load_blocked("e1f", d["e1f"], SB, H, F16, nc.sync)
            e2f_t = load_blocked("e2f", d["e2f"], S2B, H, F16, nc.sync)
            a1t_t = P.tile([128, SB * T], F16, tag="a1t")
            a2t_t = P.tile([128, S2B * T], F16, tag="a2t")

            with tc.tile_pool(name="psA", bufs=6, space="PSUM") as PSA, \
                 tc.tile_pool(name="psT", bufs=2, space="PSUM") as PST, \
                 tc.tile_pool(name="wrkA", bufs=2) as WRK, \
                 tc.tile_pool(name="stats", bufs=4) as STT:
                # sent attention, then template attention (fills PE while the
                # tail of sent softmax/transposes completes), then both C^T.
                attention(range(TB), SB, xt_hi_t, xt_lo_t, e1t_hi_t, e1t_lo_t,
                          a1t_t, d["aw1"], PSA, PST, WRK, STT, S)
                flush_last = attention(
                    range(TB), S2B, xt_hi_t, xt_lo_t, e2t_hi_t, e2t_lo_t,
                    a2t_t, d["aw2"], PSA, PST, WRK, STT, St)
                flush_last()
                c1t_t = P.tile([128, HB * T], F16, tag="e1t_hi")  # slot reuse
                ctx_matmul(c1t_t, e1f_t, a1t_t, SB, PSA)
                c2t_t = P.tile([128, HB * T], F16, tag="e1t_lo")  # slot reuse
                ctx_matmul(c2t_t, e2f_t, a2t_t, S2B, PSA)

            # fused gate + fusion stage (all f16 matmuls)
            xtf_t = load_blocked("xt_hi", d["xtf"], HB, T, F16)  # slot reuse
            with tc.tile_pool(name="psC", bufs=2, space="PSUM") as PSC, \
                 tc.tile_pool(name="wrkC", bufs=2) as WC:
                for ht in range(HB):
                    wgt = WC.tile([128, 24 * 128], F16, tag="wg")
                    nc.sync.dma_start(wgt[:], d["wg"].ap()[ht * 128:(ht + 1) * 128, :])
                    wsto = WC.tile([128, 3 * 8 * 128], F16, tag="wsto")
                    for j, wnm in enumerate(("ws", "wt", "wo")):
                        nc.sync.dma_start(
                            wsto[:, j * 1024:(j + 1) * 1024],
                            d[wnm].ap()[ht * 128:(ht + 1) * 128, :])
                    for tb in range(NT):
                        pg = PSC.tile([128, 512], F32, tag="g")
                        i = 0
                        for src in (c1t_t, c2t_t, xtf_t):
                            for kb in range(HB):
                                nc.tensor.matmul(
                                    pg[:], wgt[:, i * 128:(i + 1) * 128],
                                    src[:, kb * T + tb * 512: kb * T + tb * 512 + 512],
                                    start=(i == 0), stop=(i == 23))
                                i += 1
                        pf = []
                        for j, src in enumerate((c1t_t, c2t_t, xtf_t)):
                            pfj = PSC.tile([128, 512], F32, tag=f"f{j}")
                            for kb in range(HB):
                                nc.tensor.matmul(
                                    pfj[:],
                                    wsto[:, j * 1024 + kb * 128: j * 1024 + kb * 128 + 128],
                                    src[:, kb * T + tb * 512: kb * T + tb * 512 + 512],
                                    start=(kb == 0), stop=(kb == HB - 1))
                            pf.append(pfj)
                        g_s = WC.tile([128, 512], F32, tag="g_s")
                        nc.scalar.activation(g_s[:], pg[:], AF.Sigmoid)
                        f1_s = WC.tile([128, 512], F32, tag="f1_s")
                        nc.scalar.copy(f1_s[:], pf[0][:])
                        d_s = WC.tile([128, 512], F32, tag="d_s")
                        nc.vector.tensor_sub(d_s[:], pf[1][:], f1_s[:])
                        f13 = WC.tile([128, 512], F32, tag="f13")
                        nc.vector.tensor_add(f13[:], pf[2][:], f1_s[:])
                        t1 = WC.tile([128, 512], F32, tag="t1")
                        nc.vector.tensor_mul(t1[:], d_s[:], g_s[:])
                        t2 = WC.tile([128, 512], F32, tag="t2")
                        nc.vector.tensor_add(t2[:], t1[:], f13[:])
                        fo = WC.tile([128, 512], F32, tag="fo")
                        nc.scalar.activation(fo[:], t2[:], AF.Tanh)
                        nc.sync.dma_start(
                            d["fusT"].ap()[ht * 128:(ht + 1) * 128,
                                           tb * 512:(tb + 1) * 512], fo[:])

    nc.compile()
    return nc


def _get_nc(reps=1, bc="f16"):
    key = (reps, bc)
    if key not in _CACHE:
        _CACHE[key] = _build(reps, bc)
    return _CACHE[key]


def _prep_weight(w_t, cb, ob):
    """Pack W' [C,O] f16 so the per-output-tile lhsT DMA is contiguous.

    result[ot*128+p, kb*128+oo] = W'[kb*128+p, ot*128+oo]
    """
    return np.ascontiguousarray(
        w_t.reshape(cb, 128, ob, 128).transpose(2, 1, 0, 3).reshape(ob * 128, cb * 128))


def _hi_lo(x):
    hi = x.astype(ml_dtypes.bfloat16)
    lo = (x - hi.astype(np.float32)).astype(ml_dtypes.bfloat16)
    return hi, lo


def kernel(output, sent, template, W_gate, W_sent, W_template, W_output,
           _reps=None, _trace=False):
    from concourse.bass_utils import run_bass_kernel_spmd

    reps = _reps if _reps is not None else int(os.environ.get("BENCH_REPS", "1"))
    bc = os.environ.get("BENCH_BC", "f16")
    nc = _get_nc(reps, bc)

    f16 = np.float16 if bc == "f16" else ml_dtypes.bfloat16
    wg_p = _prep_weight(np.ascontiguousarray(W_gate.T).astype(f16), 24, 8)
    ws_p = _prep_weight(np.ascontiguousarray(W_sent.T).astype(f16), 8, 8)
    wt_p = _prep_weight(np.ascontiguousarray(W_template.T).astype(f16), 8, 8)
    wo_p = _prep_weight(np.ascontiguousarray(W_output.T).astype(f16), 8, 8)
    ident = np.eye(128, dtype=f16)

    in_maps = []
    for b in range(B):
        xt = np.ascontiguousarray(output[b].T)
        e1t = np.ascontiguousarray(sent[b].T)
        e2t = np.ascontiguousarray(template[b].T)
        xt_hi, xt_lo = _hi_lo(xt)
        e1t_hi, e1t_lo = _hi_lo(e1t)
        e2t_hi, e2t_lo = _hi_lo(e2t)
        in_maps.append({
            "xt_hi": xt_hi, "xt_lo": xt_lo,
            "e1t_hi": e1t_hi, "e1t_lo": e1t_lo,
            "e2t_hi": e2t_hi, "e2t_lo": e2t_lo,
            "e1f": sent[b].astype(f16), "e2f": template[b].astype(f16),
            "xtf": xt.astype(f16),
            "wg": wg_p, "ws": ws_p, "wt": wt_p, "wo": wo_p,
            "ident": ident,
        })

    res = run_bass_kernel_spmd(nc, in_maps, list(range(B)), trace=_trace)
    kernel.last_results = res

    fusion = np.stack([np.ascontiguousarray(res.results[b]["fusT"].T)
                       for b in range(B)])
    sent_weight = np.stack([res.results[b]["aw1"] for b in range(B)])
    template_weight = np.stack([res.results[b]["aw2"] for b in range(B)])
    return fusion, sent_weight, template_weight


# revision 56
# speedup vs baseline: 1.3101x; 1.2162x over previous
"""TRN2 Bass kernel for the two-encoder attention module.

Per batch element b (8 of them, one per NeuronCore):
    P1 = X @ E1^T          (T,S)   attention logits vs `sent`
    A1 = softmax(P1)               -> output sent_weight
    C1 = A1 @ E1           (T,H)
    P2/A2/C2 vs `template` (St)
    gate = sigmoid(cat(C1, C2, X) @ W_gate^T)        (T,H)
    fusion = tanh((1-gate)*C1@Ws^T + gate*C2@Wt^T + X@Wo^T)

Everything on-device is computed in a feature-major ("transposed") layout so
that every matmul contraction dim lands on SBUF partitions with no on-device
input transposes:
    - logits: lhsT = X^T (hi/lo bf16 split, 3-pass for fp32-grade accuracy),
      rhs = E^T (hi/lo).  Softmax along the free dim.
    - A^T obtained with PE transpose-mode matmuls (f16).
    - C^T  = E^T(natural lhsT) @ A^T    [f16]
    - gate^T, F^T via host-pretransposed/prepacked f16 weights.
Host side handles batch sharding across 8 cores, input transposes/casts and
the final un-transpose of fusion.
"""
import contextlib
import os

import ml_dtypes
import numpy as np

B, T, S, St, H = 8, 1024, 1024, 512, 1024
TB = T // 128           # 8 t-tiles
HB = H // 128           # 8 h-blocks
SB = S // 128           # 8 s-blocks (sent)
S2B = St // 128         # 4 s-blocks (template)
NT = T // 512           # 2 t-halves for 512-wide rhs

_CACHE = {}


def _build(reps=1, bc="f16", logits="hilo"):
    import concourse.bacc as bacc
    import concourse.mybir as mybir
    import concourse.tile as tile

    dt = mybir.dt
    F32, F16, BF16 = dt.float32, dt.float16, dt.bfloat16
    if bc == "bf16":
        F16 = BF16  # dtype for the value/gate/fusion stages
    LG = dt.float32r if logits == "f32r" else BF16
    AF = mybir.ActivationFunctionType
    ALU = mybir.AluOpType
    AX = mybir.AxisListType

    nc = bacc.Bacc("TRN2", target_bir_lowering=False, debug=False, num_devices=8)

    d = {}
    decls = [
        ("xt_hi", [H, T], LG),
        ("e1t_hi", [H, S], LG),
        ("e2t_hi", [H, St], LG),
        ("e1f", [S, H], F16), ("e2f", [St, H], F16), ("xtf", [H, T], F16),
        ("wg", [H, 3 * H], F16), ("ws", [H, H], F16),
        ("wt", [H, H], F16), ("wo", [H, H], F16),
        ("ident", [128, 128], F16),
    ]
    if logits == "hilo":
        decls += [("xt_lo", [H, T], BF16), ("e1t_lo", [H, S], BF16),
                  ("e2t_lo", [H, St], BF16)]
    for nm, shape, ddt in decls:
        d[nm] = nc.dram_tensor(nm, shape, ddt, kind="ExternalInput")
    d["aw1"] = nc.dram_tensor("aw1", [T, S], F32, kind="ExternalOutput")
    d["aw2"] = nc.dram_tensor("aw2", [T, St], F32, kind="ExternalOutput")
    d["fusT"] = nc.dram_tensor("fusT", [H, T], F32, kind="ExternalOutput")

    with tile.TileContext(nc) as tc, contextlib.ExitStack() as ctx:
        P = ctx.enter_context(tc.tile_pool(name="persist", bufs=1))

        ident_t = P.tile([128, 128], F16, tag="ident")
        nc.gpsimd.dma_start(ident_t[:], d["ident"].ap()[:, :])

        def load_blocked(tag, dram, nblk, width, ddt, eng=None, nsplit=1):
            t = P.tile([128, nblk * width], ddt, tag=tag)
            eng = eng or nc.sync
            cw = width // nsplit
            for sp in range(nsplit):
                for b_ in range(nblk):
                    eng.dma_start(
                        t[:, b_ * width + sp * cw: b_ * width + (sp + 1) * cw],
                        dram.ap()[b_ * 128:(b_ + 1) * 128, sp * cw:(sp + 1) * cw])
            return t

        pending = []  # (a16 tile, it, nsb, a_t) transposes deferred one block

        def attention(it_range, nsb, xt_hi_t, xt_lo_t, et_hi_t, et_lo_t, a_t,
                      aw_dram, PSA, PST, WRK, STT, width):
            """One encoder's logits+softmax+transpose. width = S or St.

            Transposes for tile `it` are emitted after the logits of `it+1`
            so the softmax chain (DVE/ACT) has a full logits-block of slack
            before the PE reaches the transpose instructions.
            """
            nhalf = width // 512

            def flush_pending():
                for a16p, itp, nsbp, a_tp in pending:
                    for sb in range(nsbp):
                        pt = PST.tile([128, 128], F16, tag="tr")
                        nc.tensor.transpose(
                            pt[:], a16p[:, sb * 128:(sb + 1) * 128], ident_t[:])
                        nc.vector.tensor_copy(
                            a_tp[:, sb * T + itp * 128: sb * T + itp * 128 + 128],
                            pt[:])
                pending.clear()

            # Tiles processed in pairs, pass-major: both tiles' (hi,hi) pass
            # runs before any (lo,*) pass, so the kernel head only waits on
            # the hi tensors' DMA; pass order matches the DMA issue order.
            if xt_lo_t is None:
                passes = ((xt_hi_t, et_hi_t),)
            else:
                passes = ((xt_hi_t, et_hi_t), (xt_lo_t, et_hi_t),
                          (xt_hi_t, et_lo_t))
            it_list = list(it_range)
            for it0 in range(0, len(it_list), 2):
                pair = it_list[it0:it0 + 2]
                ps = {it: [PSA.tile([128, 512], F32, tag="p", name=f"p{it % 2}{sh}")
                           for sh in range(nhalf)] for it in pair}
                for pi, (lt, rt) in enumerate(passes):
                    for it in pair:
                        for hb in range(HB):
                            lhsT = lt[:, hb * T + it * 128: hb * T + it * 128 + 128]
                            for sh in range(nhalf):
                                nc.tensor.matmul(
                                    ps[it][sh][:],
                                    lhsT,
                                    rt[:, hb * width + sh * 512:
                                       hb * width + sh * 512 + 512],
                                    start=(pi == 0 and hb == 0),
                                    stop=(pi == len(passes) - 1
                                          and hb == HB - 1))
                flush_pending()
                for it in pair:
                    softmax_tile(it, ps[it], nsb, a_t, aw_dram, WRK, STT, width,
                                 nhalf)
            return flush_pending

        def softmax_tile(it, ps, nsb, a_t, aw_dram, WRK, STT, width, nhalf):
                nm = STT.tile([128, 1], F32, tag="nm")
                if nhalf == 1:
                    nc.vector.tensor_reduce(nm[:], ps[0][:], axis=AX.X,
                                            op=ALU.max, negate=True)
                else:
                    nm0 = STT.tile([128, 1], F32, tag="nm0")
                    nm1 = STT.tile([128, 1], F32, tag="nm1")
                    nc.vector.tensor_reduce(nm0[:], ps[0][:], axis=AX.X,
                                            op=ALU.max, negate=True)
                    nc.vector.tensor_reduce(nm1[:], ps[1][:], axis=AX.X,
                                            op=ALU.max, negate=True)
                    nc.vector.tensor_tensor(nm[:], nm0[:], nm1[:], op=ALU.min)
                a_f32 = WRK.tile([128, width], F32, tag="a_f32")
                ssum = STT.tile([128, 1], F32, tag="ssum")
                if nhalf == 1:
                    nc.scalar.activation(a_f32[:], ps[0][:], AF.Exp,
                                         bias=nm[:], scale=1.0, accum_out=ssum[:])
                else:
                    s0 = STT.tile([128, 1], F32, tag="s0")
                    s1 = STT.tile([128, 1], F32, tag="s1")
                    nc.scalar.activation(a_f32[:, 0:512], ps[0][:], AF.Exp,
                                         bias=nm[:], scale=1.0, accum_out=s0[:])
                    nc.scalar.activation(a_f32[:, 512:1024], ps[1][:], AF.Exp,
                                         bias=nm[:], scale=1.0, accum_out=s1[:])
                    nc.vector.tensor_add(ssum[:], s0[:], s1[:])
                rinv = STT.tile([128, 1], F32, tag="rinv")
                nc.vector.reciprocal(rinv[:], ssum[:])
                an = WRK.tile([128, width], F32, tag="an")
                nc.scalar.activation(an[:], a_f32[:], AF.Copy, scale=rinv[:])
                nc.sync.dma_start(aw_dram.ap()[it * 128:(it + 1) * 128, :], an[:])
                a16 = WRK.tile([128, width], F16, tag="a16", bufs=4)
                nc.vector.tensor_scalar_mul(a16[:], a_f32[:], rinv[:])
                pending.append((a16, it, nsb, a_t))

        def ctx_matmul(c_t, ef_t, a_t, nsb, PSA):
            """C^T[k,t] = sum_s E[s,k] * A^T[s,t]. t-halves share lhsT."""
            for kt in range(HB):
                pcs = [PSA.tile([128, 512], F32, tag="p", name=f"pc{tb}")
                       for tb in range(NT)]
                for sb in range(nsb):
                    lhsT = ef_t[:, sb * H + kt * 128: sb * H + kt * 128 + 128]
                    for tb in range(NT):
                        nc.tensor.matmul(
                            pcs[tb][:], lhsT,
                            a_t[:, sb * T + tb * 512: sb * T + tb * 512 + 512],
                            start=(sb == 0), stop=(sb == nsb - 1))
                for tb in range(NT):
                    nc.scalar.copy(
                        c_t[:, kt * T + tb * 512: kt * T + tb * 512 + 512],
                        pcs[tb][:])

        for _rep in range(reps):
            pending.clear()
            # critical-path loads first: t-tile 0 logits need all of xt_hi and
            # e1t_hi; xt_lo/e1t_lo are needed one matmul-group later.
            xt_hi_t = load_blocked("xt_hi", d["xt_hi"], HB, T, LG, nc.sync)
            e1t_hi_t = load_blocked("e1t_hi", d["e1t_hi"], HB, S, LG, nc.sync)
            if logits == "hilo":
                xt_lo_t = load_blocked("xt_lo", d["xt_lo"], HB, T, BF16, nc.sync)
                e1t_lo_t = load_blocked("e1t_lo", d["e1t_lo"], HB, S, BF16,
                                        nc.sync)
            else:
                xt_lo_t = e1t_lo_t = e2t_lo_t = None
            # non-critical loads behind the criticals on the same queue, so
            # they never compete with them for HBM bandwidth at the head
            e2t_hi_t = load_blocked("e2t_hi", d["e2t_hi"], HB, St, LG, nc.sync)
            if logits == "hilo":
                e2t_lo_t = load_blocked("e2t_lo", d["e2t_lo"], HB, St, BF16,
                                        nc.sync)
            e1f_t = load_blocked("e1f", d["e1f"], SB, H, F16, nc.sync)
            e2f_t = load_blocked("e2f", d["e2f"], S2B, H, F16, nc.sync)
            a1t_t = P.tile([128, SB * T], F16, tag="a1t")
            a2t_t = P.tile([128, S2B * T], F16, tag="a2t")

            with tc.tile_pool(name="psA", bufs=6, space="PSUM") as PSA, \
                 tc.tile_pool(name="psT", bufs=2, space="PSUM") as PST, \
                 tc.tile_pool(name="wrkA", bufs=2) as WRK, \
                 tc.tile_pool(name="stats", bufs=4) as STT:
                # sent attention, then template attention (fills PE while the
                # tail of sent softmax/transposes completes), then both C^T.
                attention(range(TB), SB, xt_hi_t, xt_lo_t, e1t_hi_t, e1t_lo_t,
                          a1t_t, d["aw1"], PSA, PST, WRK, STT, S)
                flush_last = attention(
                    range(TB), S2B, xt_hi_t, xt_lo_t, e2t_hi_t, e2t_lo_t,
                    a2t_t, d["aw2"], PSA, PST, WRK, STT, St)
                flush_last()
                c1t_t = P.tile([128, HB * T], F16, tag="e1t_hi")  # slot reuse
                ctx_matmul(c1t_t, e1f_t, a1t_t, SB, PSA)
                c2t_t = P.tile([128, HB * T], F16, tag="e1t_lo")  # slot reuse
                ctx_matmul(c2t_t, e2f_t, a2t_t, S2B, PSA)

            # fused gate + fusion stage (all f16 matmuls)
            xtf_t = load_blocked("xt_hi", d["xtf"], HB, T, F16)  # slot reuse
            with tc.tile_pool(name="psC", bufs=2, space="PSUM") as PSC, \
                 tc.tile_pool(name="wrkC", bufs=2) as WC:
                for ht in range(HB):
                    wgt = WC.tile([128, 24 * 128], F16, tag="wg")
                    nc.sync.dma_start(wgt[:], d["wg"].ap()[ht * 128:(ht + 1) * 128, :])
                    wsto = WC.tile([128, 3 * 8 * 128], F16, tag="wsto")
                    for j, wnm in enumerate(("ws", "wt", "wo")):
                        nc.sync.dma_start(
                            wsto[:, j * 1024:(j + 1) * 1024],
                            d[wnm].ap()[ht * 128:(ht + 1) * 128, :])
                    for tb in range(NT):
                        pg = PSC.tile([128, 512], F32, tag="g")
                        i = 0
                        for src in (c1t_t, c2t_t, xtf_t):
                            for kb in range(HB):
                                nc.tensor.matmul(
                                    pg[:], wgt[:, i * 128:(i + 1) * 128],
                                    src[:, kb * T + tb * 512: kb * T + tb * 512 + 512],
                                    start=(i == 0), stop=(i == 23))
                                i += 1
                        pf = []
                        for j, src in enumerate((c1t_t, c2t_t, xtf_t)):
                            pfj = PSC.tile([128, 512], F32, tag=f"f{j}")
                            for kb in range(HB):
                                nc.tensor.matmul(
                                    pfj[:],
                                    wsto[:, j * 1024 + kb * 128: j * 1024 + kb * 128 + 128],
                                    src[:, kb * T + tb * 512: kb * T + tb * 512 + 512],
                                    start=(kb == 0), stop=(kb == HB - 1))
                            pf.append(pfj)
                        g_s = WC.tile([128, 512], F32, tag="g_s")
                        nc.scalar.activation(g_s[:], pg[:], AF.Sigmoid)
                        f1_s = WC.tile([128, 512], F32, tag="f1_s")
                        nc.scalar.copy(f1_s[:], pf[0][:])
                        d_s = WC.tile([128, 512], F32, tag="d_s")
                        nc.vector.tensor_sub(d_s[:], pf[1][:], f1_s[:])
                        f13 = WC.tile([128, 512], F32, tag="f13")
                        nc.vector.tensor_add(f13[:], pf[2][:], f1_s[:])
                        t1 = WC.tile([128, 512], F32, tag="t1")
                        nc.vector.tensor_mul(t1[:], d_s[:], g_s[:])
                        t2 = WC.tile([128, 512], F32, tag="t2")
                        nc.vector.tensor_add(t2[:], t1[:], f13[:])
                        fo = WC.tile([128, 512], F32, tag="fo")
                        nc.scalar.activation(fo[:], t2[:], AF.Tanh)
                        nc.sync.dma_start(
                            d["fusT"].ap()[ht * 128:(ht + 1) * 128,
                                           tb * 512:(tb + 1) * 512], fo[:])

    nc.compile()
    return nc


def _get_nc(reps=1, bc="f16", logits="hilo"):
    key = (reps, bc, logits)
    if key not in _CACHE:
        _CACHE[key] = _build(reps, bc, logits)
    return _CACHE[key]


def _prep_weight(w_t, cb, ob):
    """Pack W' [C,O] f16 so the per-output-tile lhsT DMA is contiguous.

    result[ot*128+p, kb*128+oo] = W'[kb*128+p, ot*128+oo]
    """
    return np.ascontiguousarray(
        w_t.reshape(cb, 128, ob, 128).transpose(2, 1, 0, 3).reshape(ob * 128, cb * 128))


def _hi_lo(x):
    hi = x.astype(ml_dtypes.bfloat16)
    lo = (x - hi.astype(np.float32)).astype(ml_dtypes.bfloat16)
    return hi, lo


def kernel(output, sent, template, W_gate, W_sent, W_template, W_output,
           _reps=None, _trace=False):
    from concourse.bass_utils import run_bass_kernel_spmd

    reps = _reps if _reps is not None else int(os.environ.get("BENCH_REPS", "1"))
    bc = os.environ.get("BENCH_BC", "f16")
    logits = os.environ.get("BENCH_LOGITS", "hilo")
    nc = _get_nc(reps, bc, logits)

    f16 = np.float16 if bc == "f16" else ml_dtypes.bfloat16
    wg_p = _prep_weight(np.ascontiguousarray(W_gate.T).astype(f16), 24, 8)
    ws_p = _prep_weight(np.ascontiguousarray(W_sent.T).astype(f16), 8, 8)
    wt_p = _prep_weight(np.ascontiguousarray(W_template.T).astype(f16), 8, 8)
    wo_p = _prep_weight(np.ascontiguousarray(W_output.T).astype(f16), 8, 8)
    ident = np.eye(128, dtype=f16)

    in_maps = []
    for b in range(B):
        xt = np.ascontiguousarray(output[b].T)
        e1t = np.ascontiguousarray(sent[b].T)
        e2t = np.ascontiguousarray(template[b].T)
        m = {
            "e1f": sent[b].astype(f16), "e2f": template[b].astype(f16),
            "xtf": xt.astype(f16),
            "wg": wg_p, "ws": ws_p, "wt": wt_p, "wo": wo_p,
            "ident": ident,
        }
        if logits == "hilo":
            m["xt_hi"], m["xt_lo"] = _hi_lo(xt)
            m["e1t_hi"], m["e1t_lo"] = _hi_lo(e1t)
            m["e2t_hi"], m["e2t_lo"] = _hi_lo(e2t)
        else:
            m["xt_hi"], m["e1t_hi"], m["e2t_hi"] = (
                xt.astype(np.float32), e1t.astype(np.float32),
                e2t.astype(np.float32))
        in_maps.append(m)

    res = run_bass_kernel_spmd(nc, in_maps, list(range(B)), trace=_trace)
    kernel.last_results = res

    fusion = np.stack([np.ascontiguousarray(res.results[b]["fusT"].T)
                       for b in range(B)])
    sent_weight = np.stack([res.results[b]["aw1"] for b in range(B)])
    template_weight = np.stack([res.results[b]["aw2"] for b in range(B)])
    return fusion, sent_weight, template_weight


# revision 59
# speedup vs baseline: 1.3397x; 1.0226x over previous
"""TRN2 Bass kernel for the two-encoder attention module.

Per batch element b (8 of them, one per NeuronCore):
    P1 = X @ E1^T          (T,S)   attention logits vs `sent`
    A1 = softmax(P1)               -> output sent_weight
    C1 = A1 @ E1           (T,H)
    P2/A2/C2 vs `template` (St)
    gate = sigmoid(cat(C1, C2, X) @ W_gate^T)        (T,H)
    fusion = tanh((1-gate)*C1@Ws^T + gate*C2@Wt^T + X@Wo^T)

Everything on-device is computed in a feature-major ("transposed") layout so
that every matmul contraction dim lands on SBUF partitions with no on-device
input transposes:
    - logits: lhsT = X^T (hi/lo bf16 split, 3-pass for fp32-grade accuracy),
      rhs = E^T (hi/lo).  Softmax along the free dim.
    - A^T obtained with PE transpose-mode matmuls (f16).
    - C^T  = E^T(natural lhsT) @ A^T    [f16]
    - gate^T, F^T via host-pretransposed/prepacked f16 weights.
Host side handles batch sharding across 8 cores, input transposes/casts and
the final un-transpose of fusion.
"""
import contextlib
import os

import ml_dtypes
import numpy as np

B, T, S, St, H = 8, 1024, 1024, 512, 1024
TB = T // 128           # 8 t-tiles
HB = H // 128           # 8 h-blocks
SB = S // 128           # 8 s-blocks (sent)
S2B = St // 128         # 4 s-blocks (template)
NT = T // 512           # 2 t-halves for 512-wide rhs

_CACHE = {}


def _build(reps=1, bc="f16", logits="hilo"):
    import concourse.bacc as bacc
    import concourse.mybir as mybir
    import concourse.tile as tile

    dt = mybir.dt
    F32, F16, BF16 = dt.float32, dt.float16, dt.bfloat16
    if bc == "bf16":
        F16 = BF16  # dtype for the value/gate/fusion stages
    LG = dt.float32r if logits == "f32r" else BF16
    AF = mybir.ActivationFunctionType
    ALU = mybir.AluOpType
    AX = mybir.AxisListType

    nc = bacc.Bacc("TRN2", target_bir_lowering=False, debug=False, num_devices=8)

    d = {}
    decls = [
        ("xt_hi", [H, T], LG),
        ("e1t_hi", [H, S], LG),
        ("e2t_hi", [H, St], LG),
        ("e1f", [S, H], F16), ("e2f", [St, H], F16), ("xtf", [H, T], F16),
        ("wg", [H, 3 * H], F16), ("ws", [H, H], F16),
        ("wt", [H, H], F16), ("wo", [H, H], F16),
        ("ident", [128, 128], F16),
    ]
    if logits == "hilo":
        decls += [("xt_lo", [H, T], BF16), ("e1t_lo", [H, S], BF16),
                  ("e2t_lo", [H, St], BF16)]
    for nm, shape, ddt in decls:
        d[nm] = nc.dram_tensor(nm, shape, ddt, kind="ExternalInput")
    d["aw1"] = nc.dram_tensor("aw1", [T, S], F32, kind="ExternalOutput")
    d["aw2"] = nc.dram_tensor("aw2", [T, St], F32, kind="ExternalOutput")
    d["fusT"] = nc.dram_tensor("fusT", [H, T], F32, kind="ExternalOutput")

    with tile.TileContext(nc) as tc, contextlib.ExitStack() as ctx:
        P = ctx.enter_context(tc.tile_pool(name="persist", bufs=1))

        ident_t = P.tile([128, 128], F16, tag="ident")
        nc.gpsimd.dma_start(ident_t[:], d["ident"].ap()[:, :])

        def load_blocked(tag, dram, nblk, width, ddt, eng=None, nsplit=1):
            t = P.tile([128, nblk * width], ddt, tag=tag)
            eng = eng or nc.sync
            cw = width // nsplit
            for sp in range(nsplit):
                for b_ in range(nblk):
                    eng.dma_start(
                        t[:, b_ * width + sp * cw: b_ * width + (sp + 1) * cw],
                        dram.ap()[b_ * 128:(b_ + 1) * 128, sp * cw:(sp + 1) * cw])
            return t

        pending = []  # (a16 tile, it, nsb, a_t) transposes deferred one block

        def attention(it_range, nsb, xt_hi_t, xt_lo_t, et_hi_t, et_lo_t, a_t,
                      aw_dram, PSA, PST, WRK, STT, width):
            """One encoder's logits+softmax+transpose. width = S or St.

            Transposes for tile `it` are emitted after the logits of `it+1`
            so the softmax chain (DVE/ACT) has a full logits-block of slack
            before the PE reaches the transpose instructions.
            """
            nhalf = width // 512

            def flush_pending():
                for a16p, itp, nsbp, a_tp in pending:
                    for sb in range(nsbp):
                        pt = PST.tile([128, 128], F16, tag="tr")
                        nc.tensor.transpose(
                            pt[:], a16p[:, sb * 128:(sb + 1) * 128], ident_t[:])
                        nc.vector.tensor_copy(
                            a_tp[:, sb * T + itp * 128: sb * T + itp * 128 + 128],
                            pt[:])
                pending.clear()

            # Tiles processed in pairs, pass-major: both tiles' (hi,hi) pass
            # runs before any (lo,*) pass, so the kernel head only waits on
            # the hi tensors' DMA; pass order matches the DMA issue order.
            if xt_lo_t is None:
                passes = ((xt_hi_t, et_hi_t),)
            else:
                passes = ((xt_hi_t, et_hi_t), (xt_lo_t, et_hi_t),
                          (xt_hi_t, et_lo_t))
            it_list = list(it_range)
            for it0 in range(0, len(it_list), 2):
                pair = it_list[it0:it0 + 2]
                ps = {it: [PSA.tile([128, 512], F32, tag="p", name=f"p{it % 2}{sh}")
                           for sh in range(nhalf)] for it in pair}
                if len(passes) == 1:
                    # single-pass logits: s-half-outer so the first groups
                    # only need the first half of e^T from DRAM
                    lt, rt = passes[0]
                    for sh in range(nhalf):
                        for it in pair:
                            for hb in range(HB):
                                nc.tensor.matmul(
                                    ps[it][sh][:],
                                    lt[:, hb * T + it * 128:
                                       hb * T + it * 128 + 128],
                                    rt[:, hb * width + sh * 512:
                                       hb * width + sh * 512 + 512],
                                    start=(hb == 0), stop=(hb == HB - 1))
                else:
                    for pi, (lt, rt) in enumerate(passes):
                        for it in pair:
                            for hb in range(HB):
                                lhsT = lt[:, hb * T + it * 128:
                                          hb * T + it * 128 + 128]
                                for sh in range(nhalf):
                                    nc.tensor.matmul(
                                        ps[it][sh][:],
                                        lhsT,
                                        rt[:, hb * width + sh * 512:
                                           hb * width + sh * 512 + 512],
                                        start=(pi == 0 and hb == 0),
                                        stop=(pi == len(passes) - 1
                                              and hb == HB - 1))
                flush_pending()
                for it in pair:
                    softmax_tile(it, ps[it], nsb, a_t, aw_dram, WRK, STT, width,
                                 nhalf)
            return flush_pending

        def softmax_tile(it, ps, nsb, a_t, aw_dram, WRK, STT, width, nhalf):
                nm = STT.tile([128, 1], F32, tag="nm")
                if nhalf == 1:
                    nc.vector.tensor_reduce(nm[:], ps[0][:], axis=AX.X,
                                            op=ALU.max, negate=True)
                else:
                    nm0 = STT.tile([128, 1], F32, tag="nm0")
                    nm1 = STT.tile([128, 1], F32, tag="nm1")
                    nc.vector.tensor_reduce(nm0[:], ps[0][:], axis=AX.X,
                                            op=ALU.max, negate=True)
                    nc.vector.tensor_reduce(nm1[:], ps[1][:], axis=AX.X,
                                            op=ALU.max, negate=True)
                    nc.vector.tensor_tensor(nm[:], nm0[:], nm1[:], op=ALU.min)
                a_f32 = WRK.tile([128, width], F32, tag="a_f32")
                ssum = STT.tile([128, 1], F32, tag="ssum")
                if nhalf == 1:
                    nc.scalar.activation(a_f32[:], ps[0][:], AF.Exp,
                                         bias=nm[:], scale=1.0, accum_out=ssum[:])
                else:
                    s0 = STT.tile([128, 1], F32, tag="s0")
                    s1 = STT.tile([128, 1], F32, tag="s1")
                    nc.scalar.activation(a_f32[:, 0:512], ps[0][:], AF.Exp,
                                         bias=nm[:], scale=1.0, accum_out=s0[:])
                    nc.scalar.activation(a_f32[:, 512:1024], ps[1][:], AF.Exp,
                                         bias=nm[:], scale=1.0, accum_out=s1[:])
                    nc.vector.tensor_add(ssum[:], s0[:], s1[:])
                rinv = STT.tile([128, 1], F32, tag="rinv")
                nc.vector.reciprocal(rinv[:], ssum[:])
                an = WRK.tile([128, width], F32, tag="an")
                nc.scalar.activation(an[:], a_f32[:], AF.Copy, scale=rinv[:])
                nc.sync.dma_start(aw_dram.ap()[it * 128:(it + 1) * 128, :], an[:])
                a16 = WRK.tile([128, width], F16, tag="a16", bufs=4)
                nc.vector.tensor_scalar_mul(a16[:], a_f32[:], rinv[:])
                pending.append((a16, it, nsb, a_t))

        def ctx_matmul(c_t, ef_t, a_t, nsb, PSA):
            """C^T[k,t] = sum_s E[s,k] * A^T[s,t]. t-halves share lhsT."""
            for kt in range(HB):
                pcs = [PSA.tile([128, 512], F32, tag="p", name=f"pc{tb}")
                       for tb in range(NT)]
                for sb in range(nsb):
                    lhsT = ef_t[:, sb * H + kt * 128: sb * H + kt * 128 + 128]
                    for tb in range(NT):
                        nc.tensor.matmul(
                            pcs[tb][:], lhsT,
                            a_t[:, sb * T + tb * 512: sb * T + tb * 512 + 512],
                            start=(sb == 0), stop=(sb == nsb - 1))
                for tb in range(NT):
                    nc.scalar.copy(
                        c_t[:, kt * T + tb * 512: kt * T + tb * 512 + 512],
                        pcs[tb][:])

        for _rep in range(reps):
            pending.clear()
            # critical-path loads first: t-tile 0 logits need all of xt_hi and
            # e1t_hi; xt_lo/e1t_lo are needed one matmul-group later.
            if logits == "f32r":
                # interleave column-half chunks in the order the first two
                # tile-pairs consume them: xt-c0, e1t-h0, e1t-h1, xt-c1
                xt_hi_t = P.tile([128, HB * T], LG, tag="xt_hi")
                e1t_hi_t = P.tile([128, HB * S], LG, tag="e1t_hi")
                for t_, dram, wid, sp in ((xt_hi_t, d["xt_hi"], T, 0),
                                          (e1t_hi_t, d["e1t_hi"], S, 0),
                                          (e1t_hi_t, d["e1t_hi"], S, 1),
                                          (xt_hi_t, d["xt_hi"], T, 1)):
                    for b_ in range(HB):
                        nc.sync.dma_start(
                            t_[:, b_ * wid + sp * 512: b_ * wid + sp * 512 + 512],
                            dram.ap()[b_ * 128:(b_ + 1) * 128,
                                      sp * 512:(sp + 1) * 512])
            else:
                xt_hi_t = load_blocked("xt_hi", d["xt_hi"], HB, T, LG, nc.sync)
                e1t_hi_t = load_blocked("e1t_hi", d["e1t_hi"], HB, S, LG, nc.sync)
            if logits == "hilo":
                xt_lo_t = load_blocked("xt_lo", d["xt_lo"], HB, T, BF16, nc.sync)
                e1t_lo_t = load_blocked("e1t_lo", d["e1t_lo"], HB, S, BF16,
                                        nc.sync)
            else:
                xt_lo_t = e1t_lo_t = e2t_lo_t = None
            # non-critical loads behind the criticals on the same queue, so
            # they never compete with them for HBM bandwidth at the head
            e2t_hi_t = load_blocked("e2t_hi", d["e2t_hi"], HB, St, LG, nc.sync)
            if logits == "hilo":
                e2t_lo_t = load_blocked("e2t_lo", d["e2t_lo"], HB, St, BF16,
                                        nc.sync)
            e1f_t = load_blocked("e1f", d["e1f"], SB, H, F16, nc.sync)
            e2f_t = load_blocked("e2f", d["e2f"], S2B, H, F16, nc.sync)
            a1t_t = P.tile([128, SB * T], F16, tag="a1t")
            a2t_t = P.tile([128, S2B * T], F16, tag="a2t")

            with tc.tile_pool(name="psA", bufs=6, space="PSUM") as PSA, \
                 tc.tile_pool(name="psT", bufs=2, space="PSUM") as PST, \
                 tc.tile_pool(name="wrkA", bufs=2) as WRK, \
                 tc.tile_pool(name="stats", bufs=4) as STT:
                # sent attention, then template attention (fills PE while the
                # tail of sent softmax/transposes completes), then both C^T.
                attention(range(TB), SB, xt_hi_t, xt_lo_t, e1t_hi_t, e1t_lo_t,
                          a1t_t, d["aw1"], PSA, PST, WRK, STT, S)
                flush_last = attention(
                    range(TB), S2B, xt_hi_t, xt_lo_t, e2t_hi_t, e2t_lo_t,
                    a2t_t, d["aw2"], PSA, PST, WRK, STT, St)
                flush_last()
                c1t_t = P.tile([128, HB * T], F16, tag="e1t_hi")  # slot reuse
                ctx_matmul(c1t_t, e1f_t, a1t_t, SB, PSA)
                c2t_t = P.tile([128, HB * T], F16, tag="e1t_lo")  # slot reuse
                ctx_matmul(c2t_t, e2f_t, a2t_t, S2B, PSA)

            # fused gate + fusion stage (all f16 matmuls)
            xtf_t = load_blocked("xt_hi", d["xtf"], HB, T, F16)  # slot reuse
            with tc.tile_pool(name="psC", bufs=2, space="PSUM") as PSC, \
                 tc.tile_pool(name="wrkC", bufs=2) as WC:
                for ht in range(HB):
                    wgt = WC.tile([128, 24 * 128], F16, tag="wg")
                    nc.sync.dma_start(wgt[:], d["wg"].ap()[ht * 128:(ht + 1) * 128, :])
                    wsto = WC.tile([128, 3 * 8 * 128], F16, tag="wsto")
                    for j, wnm in enumerate(("ws", "wt", "wo")):
                        nc.sync.dma_start(
                            wsto[:, j * 1024:(j + 1) * 1024],
                            d[wnm].ap()[ht * 128:(ht + 1) * 128, :])
                    for tb in range(NT):
                        pg = PSC.tile([128, 512], F32, tag="g")
                        i = 0
                        for src in (c1t_t, c2t_t, xtf_t):
                            for kb in range(HB):
                                nc.tensor.matmul(
                                    pg[:], wgt[:, i * 128:(i + 1) * 128],
                                    src[:, kb * T + tb * 512: kb * T + tb * 512 + 512],
                                    start=(i == 0), stop=(i == 23))
                                i += 1
                        pf = []
                        for j, src in enumerate((c1t_t, c2t_t, xtf_t)):
                            pfj = PSC.tile([128, 512], F32, tag=f"f{j}")
                            for kb in range(HB):
                                nc.tensor.matmul(
                                    pfj[:],
                                    wsto[:, j * 1024 + kb * 128: j * 1024 + kb * 128 + 128],
                                    src[:, kb * T + tb * 512: kb * T + tb * 512 + 512],
                                    start=(kb == 0), stop=(kb == HB - 1))
                            pf.append(pfj)
                        g_s = WC.tile([128, 512], F32, tag="g_s")
                        nc.scalar.activation(g_s[:], pg[:], AF.Sigmoid)
                        f1_s = WC.tile([128, 512], F32, tag="f1_s")
                        nc.scalar.copy(f1_s[:], pf[0][:])
                        d_s = WC.tile([128, 512], F32, tag="d_s")
                        nc.vector.tensor_sub(d_s[:], pf[1][:], f1_s[:])
                        f13 = WC.tile([128, 512], F32, tag="f13")
                        nc.vector.tensor_add(f13[:], pf[2][:], f1_s[:])
                        t1 = WC.tile([128, 512], F32, tag="t1")
                        nc.vector.tensor_mul(t1[:], d_s[:], g_s[:])
                        t2 = WC.tile([128, 512], F32, tag="t2")
                        nc.vector.tensor_add(t2[:], t1[:], f13[:])
                        fo = WC.tile([128, 512], F32, tag="fo")
                        nc.scalar.activation(fo[:], t2[:], AF.Tanh)
                        nc.sync.dma_start(
                            d["fusT"].ap()[ht * 128:(ht + 1) * 128,
                                           tb * 512:(tb + 1) * 512], fo[:])

    nc.compile()
    return nc


def _get_nc(reps=1, bc="f16", logits="hilo"):
    key = (reps, bc, logits)
    if key not in _CACHE:
        _CACHE[key] = _build(reps, bc, logits)
    return _CACHE[key]


def _prep_weight(w_t, cb, ob):
    """Pack W' [C,O] f16 so the per-output-tile lhsT DMA is contiguous.

    result[ot*128+p, kb*128+oo] = W'[kb*128+p, ot*128+oo]
    """
    return np.ascontiguousarray(
        w_t.reshape(cb, 128, ob, 128).transpose(2, 1, 0, 3).reshape(ob * 128, cb * 128))


def _hi_lo(x):
    hi = x.astype(ml_dtypes.bfloat16)
    lo = (x - hi.astype(np.float32)).astype(ml_dtypes.bfloat16)
    return hi, lo


def kernel(output, sent, template, W_gate, W_sent, W_template, W_output,
           _reps=None, _trace=False):
    from concourse.bass_utils import run_bass_kernel_spmd

    reps = _reps if _reps is not None else int(os.environ.get("BENCH_REPS", "1"))
    bc = os.environ.get("BENCH_BC", "f16")
    logits = os.environ.get("BENCH_LOGITS", "f32r")
    nc = _get_nc(reps, bc, logits)

    f16 = np.float16 if bc == "f16" else ml_dtypes.bfloat16
    wg_p = _prep_weight(np.ascontiguousarray(W_gate.T).astype(f16), 24, 8)
    ws_p = _prep_weight(np.ascontiguousarray(W_sent.T).astype(f16), 8, 8)
    wt_p = _prep_weight(np.ascontiguousarray(W_template.T).astype(f16), 8, 8)
    wo_p = _prep_weight(np.ascontiguousarray(W_output.T).astype(f16), 8, 8)
    ident = np.eye(128, dtype=f16)

    in_maps = []
    for b in range(B):
        xt = np.ascontiguousarray(output[b].T)
        e1t = np.ascontiguousarray(sent[b].T)
        e2t = np.ascontiguousarray(template[b].T)
        m = {
            "e1f": sent[b].astype(f16), "e2f": template[b].astype(f16),
            "xtf": xt.astype(f16),
            "wg": wg_p, "ws": ws_p, "wt": wt_p, "wo": wo_p,
            "ident": ident,
        }
        if logits == "hilo":
            m["xt_hi"], m["xt_lo"] = _hi_lo(xt)
            m["e1t_hi"], m["e1t_lo"] = _hi_lo(e1t)
            m["e2t_hi"], m["e2t_lo"] = _hi_lo(e2t)
        else:
            m["xt_hi"], m["e1t_hi"], m["e2t_hi"] = (
                xt.astype(np.float32), e1t.astype(np.float32),
                e2t.astype(np.float32))
        in_maps.append(m)

    res = run_bass_kernel_spmd(nc, in_maps, list(range(B)), trace=_trace)
    kernel.last_results = res

    fusion = np.stack([np.ascontiguousarray(res.results[b]["fusT"].T)
                       for b in range(B)])
    sent_weight = np.stack([res.results[b]["aw1"] for b in range(B)])
    template_weight = np.stack([res.results[b]["aw2"] for b in range(B)])
    return fusion, sent_weight, template_weight
